# revision 34
# baseline (speedup 1.0000x reference)
"""Multi-head cross-attention Trainium2 Bass kernel, SPMD over 8 NeuronCores.

Sharding: core c handles batch b = c//2 and head group g = c%2 (8 of 16 heads).
Each core computes a partial output projection (its heads' W_o rows); the host
sums the two partials per batch element.

Device pipeline per core (all matmuls bf16 with fp32 PSUM accumulation):
  kT = (Wk^T x^T)          [512 hd, 2048 kseq]   (per-partition bias b_k)
  v  = (x Wv)              [2048 kseq, 8*65]     (65th col per head = ones)
  qT = (Wq^T y^T)          [512 hd, 1024 q]      (per-partition bias b_q)
  per (head-pair, q-tile, k-chunk):
      S^T[k, q|q'] = kT_h^T-chunk @ qT_h for both heads of the pair
        (K=64 row-tiled at partitions 0/64 -> the two matmuls run
         concurrently in the PE array; both write one 2-bank PSUM tile)
      em = exp(0.125 * S^T)  (one ACT op per pair; no row-max: |S|<=~25)
      em *= maskT            (one DVE mul per pair; mask chunk read once via
                              a step-0 broadcast dim)
      acc_h[65, q] += [v_h | 1]^T @ em_h   (row 64 = softmax denominator)
        -- the PV pair TRAILS its scores by 3 steps with projection filler
           emitted in between, so the PE queue (strict FIFO) never stalls
           on the exp->mask chain latency (~1.9us).
  normalize: vals_h = acc[0:64] * bcast(1/acc[64])  (PE outer-product bcast,
      reciprocal_approx_fast; deferred into later steps; softmax-drain
      copies run on DVE so ACT carries nothing but the exp stream)
  out_partial = vals^T-chunks @ Wo-rows  -> [1024 q, 1024 D] fp32
All host-staged inputs are laid out so every DMA source row is a multi-KB
contiguous run (1KB packets cap the DMA engines at ~200 GB/s; 8KB packets
saturate ~360 GB/s), and the first-needed tensors issue first across BOTH
HWDGE queues (sync + scalar) so the first kT chain starts ~12us in.
b_v and b_o fold into a host-side constant row (attn rows sum to 1).
"""

import sys
from collections import deque

import numpy as np
import ml_dtypes

if "/opt/trn_rl_repo" not in sys.path:
    sys.path.insert(0, "/opt/trn_rl_repo")

BF = ml_dtypes.bfloat16

B, NKV, NQ, D, H = 4, 2048, 1024, 1024, 16
HD = D // H          # 64
NHL = 8              # heads per core (local)
P = 128
DC = D // P          # 8 contraction chunks over model dim
KC = NKV // P        # 16 key-seq chunks
QT = NQ // 512       # 2 q tiles of 512 for attention
MT = 4               # hd-dim chunks of kT/qT (512/128)

_CACHE = {}


def _build_program():
    import concourse.bass as bass
    import concourse.mybir as mybir
    import concourse.tile as tile
    from concourse import bacc

    f32 = mybir.dt.float32
    bf16 = mybir.dt.bfloat16

    nc = bacc.Bacc(
        "TRN2", target_bir_lowering=False, debug=False, num_devices=8
    )

    # Host staging puts every tensor in the exact SBUF layout so each DMA
    # source ROW is one long contiguous run (-> 8KB packets, ~2x the DMA
    # throughput of the 1KB-segment layouts).
    xT_d = nc.dram_tensor("xT", [4 * P, DC * 512], bf16, kind="ExternalInput").ap()
    yT_d = nc.dram_tensor("yT", [2 * P, DC * 512], bf16, kind="ExternalInput").ap()
    maskT_d = nc.dram_tensor("maskT", [NKV, NQ], bf16, kind="ExternalInput").ap()
    wk_d = nc.dram_tensor("wk", [P, DC * 512], bf16, kind="ExternalInput").ap()
    wv_d = nc.dram_tensor("wv", [P, DC * 512], bf16, kind="ExternalInput").ap()
    wq_d = nc.dram_tensor("wq", [P, DC * 512], bf16, kind="ExternalInput").ap()
    wo_d = nc.dram_tensor("wo", [P, MT * D], bf16, kind="ExternalInput").ap()
    bk_d = nc.dram_tensor("bk", [P, MT], f32, kind="ExternalInput").ap()
    bq_d = nc.dram_tensor("bq", [P, MT], f32, kind="ExternalInput").ap()
    out_d = nc.dram_tensor("out", [NQ, D], bf16, kind="ExternalOutput").ap()

    Exp = mybir.ActivationFunctionType.Exp

    with tile.TileContext(nc) as tc:
        with (
            tc.tile_pool(name="persist", bufs=1) as persist,
            tc.tile_pool(name="work", bufs=3) as work,
            tc.tile_pool(name="empool", bufs=9) as empool,
            tc.tile_pool(name="pmm", bufs=2, space="PSUM") as pmm,
            tc.tile_pool(name="pacc", bufs=2, space="PSUM") as pacc,
            tc.tile_pool(name="psc", bufs=2, space="PSUM") as psc,
        ):
            wk_big = persist.tile([P, DC * 512], bf16, tag="wk", name="wk")
            wv_big = persist.tile([P, DC * 512], bf16, tag="wv", name="wv")
            wq_big = persist.tile([P, DC * 512], bf16, tag="wq", name="wq")
            wo_big = persist.tile([P, MT * D], bf16, tag="wo", name="wo")
            bk_big = persist.tile([P, MT], f32, tag="bk", name="bk")
            bq_big = persist.tile([P, MT], f32, tag="bq", name="bq")
            mT_big = persist.tile([P, KC * NQ], bf16, tag="mT", name="mT")
            xT_big = persist.tile([P, 4 * DC * 512], bf16, tag="xT", name="xT")
            yT_big = persist.tile([P, 2 * DC * 512], bf16, tag="yT", name="yT")

            wk_sb = [wk_big[:, d * 512:(d + 1) * 512] for d in range(DC)]
            wv_sb = [wv_big[:, d * 512:(d + 1) * 512] for d in range(DC)]
            wq_sb = [wq_big[:, d * 512:(d + 1) * 512] for d in range(DC)]
            wo_sb = [wo_big[:, c * D:(c + 1) * D] for c in range(MT)]
            bk_sb = [bk_big[:, m:m + 1] for m in range(MT)]
            bq_sb = [bq_big[:, m:m + 1] for m in range(MT)]
            maskT_sb = [mT_big[:, k * NQ:(k + 1) * NQ] for k in range(KC)]

            def xq(qb, d):   # x^T quarter qb (kseq cols 512qb..), d-chunk
                o = (qb * DC + d) * 512
                return xT_big[:, o:o + 512]

            def yq(nb, d):   # y^T half nb (q cols 512nb..), d-chunk
                o = (nb * DC + d) * 512
                return yT_big[:, o:o + 512]

            # ---- input DMAs: first-needed first, spread across the two
            # HWDGE queues (sync + scalar) so issue costs overlap ----
            def load_xt_q(qb, eng):
                eng.dma_start(
                    xT_big[:, qb * DC * 512:(qb + 1) * DC * 512],
                    xT_d[qb * P:(qb + 1) * P, :],
                )

            def load_yt_h(nb, eng):
                eng.dma_start(
                    yT_big[:, nb * DC * 512:(nb + 1) * DC * 512],
                    yT_d[nb * P:(nb + 1) * P, :],
                )

            def load_mask(k0, k1):
                dst = mT_big.rearrange("p (k c) -> p k c", k=KC)[:, k0:k1, :]
                src = maskT_d.rearrange("(k p) c -> p k c", k=KC, p=P)[
                    :, k0:k1, :]
                nc.sync.dma_start(dst, src)

            # first-needed tensors in halves so the first kT chain's d=0..3
            # matmuls unblock as soon as half the data lands
            HW2 = DC * 512 // 2
            nc.sync.dma_start(xT_big[:, 0:HW2], xT_d[0:P, 0:HW2])
            nc.scalar.dma_start(wk_big[:, 0:HW2], wk_d[:, 0:HW2])
            nc.sync.dma_start(xT_big[:, HW2:2 * HW2], xT_d[0:P, HW2:2 * HW2])
            nc.scalar.dma_start(wk_big[:, HW2:2 * HW2], wk_d[:, HW2:2 * HW2])
            nc.sync.dma_start(wv_big[:, 0:HW2], wv_d[:, 0:HW2])
            nc.sync.dma_start(wv_big[:, HW2:2 * HW2], wv_d[:, HW2:2 * HW2])
            nc.scalar.dma_start(wq_big, wq_d)
            load_yt_h(0, nc.sync)
            nc.scalar.dma_start(bk_big, bk_d)
            nc.scalar.dma_start(bq_big, bq_d)
            load_mask(0, 2)
            nc.scalar.dma_start(
                yT_big[:, DC * 512:2 * DC * 512], yT_d[P:2 * P, :])
            load_xt_q(1, nc.sync)
            load_mask(2, 4)
            load_mask(4, 8)
            load_xt_q(2, nc.sync)
            load_mask(8, 12)
            load_xt_q(3, nc.sync)
            load_mask(12, KC)
            nc.sync.dma_start(wo_big, wo_d)

            ones_sb = persist.tile([1, HD], bf16, tag="ones", name="ones")
            nc.gpsimd.memset(ones_sb, 1.0)

            kT_sb = [
                persist.tile([P, NKV], bf16, tag=f"kT{m}", name=f"kT{m}")
                for m in range(MT)
            ]
            qT_sb = [
                persist.tile([P, NQ], bf16, tag=f"qT{m}", name=f"qT{m}")
                for m in range(MT)
            ]
            v_sb = [
                persist.tile([P, NHL * 65], bf16, tag=f"v{i}", name=f"v{i}")
                for i in range(KC)
            ]
            for i in range(KC):
                nc.gpsimd.memset(
                    v_sb[i].rearrange("p (h c) -> p h c", c=65)[:, :, 64:65], 1.0
                )
            vals_sb = [
                persist.tile([P, NQ], bf16, tag=f"vals{c}", name=f"vals{c}")
                for c in range(MT)
            ]

            # ---- projection op queues (kT/qT chunk m as a list of closures,
            # one matmul each; the chain's last op appends the bias-add) ----
            def proj_ops(m, which, ns=None):
                w_sb, dst, bias, ncols = (
                    (wk_sb, kT_sb, bk_sb, NKV) if which == "k"
                    else (wq_sb, qT_sb, bq_sb, NQ)
                )
                ops = []
                hold = {}
                nlist = list(range(ncols // 512) if ns is None else ns)
                for n in nlist:
                    for d in range(DC):
                        def op(m=m, n=n, d=d, w_sb=w_sb, dst=dst, bias=bias,
                               which=which):
                            if d == 0:
                                hold[n] = pmm.tile(
                                    [P, 512], f32, tag="mm",
                                    name=f"pj{which}{m}_{n}"
                                )
                            nc.tensor.matmul(
                                hold[n],
                                lhsT=w_sb[d][:, m * P:(m + 1) * P],
                                rhs=(xq if which == "k" else yq)(n, d),
                                start=(d == 0),
                                stop=(d == DC - 1),
                            )
                            if d == DC - 1:
                                nc.vector.tensor_scalar_add(
                                    dst[m][:, n * 512:(n + 1) * 512],
                                    hold[n], bias[m]
                                )
                        ops.append(op)
                return ops

            def emit_v(i):
                ps_v = pmm.tile([P, 512], f32, tag="mm", name=f"ps_v{i}")
                for d in range(DC):
                    nc.tensor.matmul(
                        ps_v,
                        lhsT=xq(i // 4, d)[:, (i % 4) * P:(i % 4 + 1) * P],
                        rhs=wv_sb[d],
                        start=(d == 0),
                        stop=(d == DC - 1),
                    )
                v3 = v_sb[i].rearrange("p (h c) -> p h c", c=65)
                nc.vector.tensor_copy(
                    v3[:, :, 0:64], ps_v.rearrange("p (h c) -> p h c", c=64)
                )

            # ---- attention: scores/exp/mask emitted per step; the PV pair
            # trails by 2 steps (filler fills the gap) ----
            pv_backlog = deque()   # entries: (kc, closure)
            norm_pending = deque()

            normb_pending = deque()

            def make_norm(hp, t, a, h, ut, s_f):
                po = a * HD
                qs = slice(t * 512, (t + 1) * 512)
                # stage a: reciprocal + SBUF->SBUF partition-broadcast DMA on
                # the idle sync queue (replaces a PE outer-product matmul);
                # stage b (the all-SBUF bf16 multiply, 2x DVE mode) pops a few
                # steps later so the DVE never waits on the DMA latency

                def norm_a():
                    r_f = work.tile([1, 512], f32, tag="r", name=f"r{h}_{t}",
                                    bufs=2)
                    nc.vector.reciprocal_approx_fast(r_f, s_f)
                    r_b = work.tile([1, 512], bf16, tag="rb",
                                    name=f"rb{h}_{t}", bufs=2)
                    nc.vector.tensor_copy(r_b, r_f)
                    rb_bc = work.tile([HD, 512], bf16, tag="rbb",
                                      name=f"rbb{h}_{t}", bufs=2)
                    nc.sync.dma_start(
                        rb_bc,
                        r_b.rearrange("p (o q) -> p o q", o=1)
                        .broadcast_to([1, HD, 512]),
                    )

                    def norm_b():
                        nc.vector.tensor_mul(
                            vals_sb[hp][po:po + HD, qs], ut, rb_bc)
                    normb_pending.append(norm_b)
                return norm_a

            def drain_accs(hp, t, accs, h0, h1):
                for a, h in enumerate((h0, h1)):
                    acc = accs[a]
                    # free the PSUM accumulator quickly; all drain copies run
                    # on DVE so the ACT queue carries nothing but exps
                    ut = work.tile(
                        [HD, 512], bf16, tag="ut", name=f"ut{h}_{t}", bufs=4
                    )
                    nc.vector.tensor_copy(ut, acc[0:HD, :])
                    s_f = work.tile(
                        [1, 512], f32, tag="s", name=f"s{h}_{t}", bufs=5
                    )
                    nc.vector.tensor_copy(s_f, acc[64:65, :])
                    norm_pending.append(make_norm(hp, t, a, h, ut, s_f))

            def attn_scores(hp, t, kc, h0, h1, accs_box):
                qs = slice(t * 512, (t + 1) * 512)
                sp2 = psc.tile(
                    [P, 1024], f32, tag="sc", name=f"sp{hp}_{t}_{kc}"
                )
                for a in range(2):
                    po = a * HD
                    nc.tensor.matmul(
                        sp2[:, a * 512:(a + 1) * 512],
                        lhsT=kT_sb[hp][po:po + HD, kc * P:(kc + 1) * P],
                        rhs=qT_sb[hp][po:po + HD, qs],
                        start=True,
                        stop=True,
                    )
                em2 = empool.tile(
                    [P, 1024], bf16, tag="em", name=f"em{hp}_{t}_{kc}"
                )
                nc.scalar.activation(em2, sp2, Exp, scale=0.125)
                mb = (maskT_sb[kc][:, qs]
                      .rearrange("p (o q) -> p o q", o=1)
                      .broadcast_to([P, 2, 512]))
                em3 = em2.rearrange("p (o q) -> p o q", o=2)
                nc.vector.tensor_mul(em3, em3, mb)

                def pv(hp=hp, t=t, kc=kc):
                    if kc == 0:
                        accs_box[:] = [
                            pacc.tile([65, 512], f32, tag="acc",
                                      name=f"acc{h}_{t}")
                            for h in (h0, h1)
                        ]
                    for a, h in enumerate((h0, h1)):
                        nc.tensor.matmul(
                            accs_box[a],
                            lhsT=v_sb[kc][:, h * 65:(h + 1) * 65],
                            rhs=em2[:, a * 512:(a + 1) * 512],
                            start=(kc == 0),
                            stop=(kc == KC - 1),
                        )
                    if kc == KC - 1:
                        drain_accs(hp, t, accs_box, h0, h1)
                pv_backlog.append((kc, pv))

            # ---- deferred projections, drained as per-step filler ----
            pending = {
                1: deque(proj_ops(1, "q", ns=[0])
                         + proj_ops(1, "k", ns=[2, 3])
                         + proj_ops(1, "q", ns=[1])),
                2: deque(proj_ops(2, "k")),
                3: deque(proj_ops(2, "q")),
                4: deque(proj_ops(3, "k", ns=[0, 1, 2])),
                5: deque(proj_ops(3, "q")),
                6: deque(proj_ops(3, "k", ns=[3])),
                7: deque(),
            }

            def pump_pv(depth=2):
                while len(pv_backlog) > depth:
                    pv_backlog.popleft()[1]()

            def wo_units(t2s, pool_pick):
                # output-projection chains; ot tiles are [P, 1024] so the
                # stores are single fat DMAs; PSUM->SBUF copies alternate
                # ACT/DVE; store issues alternate sync/scalar queues
                ops = []
                hold = {}
                for t2 in t2s:
                    for n in range(2):
                        for c in range(MT):
                            def op(t2=t2, n=n, c=c):
                                if c == 0:
                                    pool, tag = pool_pick(t2, n)
                                    hold[(t2, n)] = pool.tile(
                                        [P, 512], f32, tag=tag,
                                        name=f"ps_o{t2}_{n}"
                                    )
                                    if ("ot", t2) not in hold:
                                        hold[("ot", t2)] = work.tile(
                                            [P, 1024], bf16, tag="ot",
                                            name=f"ot{t2}", bufs=3
                                        )
                                        hold[("done", t2)] = 0
                                ps_o = hold[(t2, n)]
                                nc.tensor.matmul(
                                    ps_o,
                                    lhsT=vals_sb[c][:, t2 * P:(t2 + 1) * P],
                                    rhs=wo_sb[c][:, n * 512:(n + 1) * 512],
                                    start=(c == 0),
                                    stop=(c == MT - 1),
                                )
                                if c == MT - 1:
                                    ot = hold[("ot", t2)]
                                    dst = ot[:, n * 512:(n + 1) * 512]
                                    if (t2 + n) % 2 == 0:
                                        nc.scalar.copy(dst, ps_o)
                                    else:
                                        nc.vector.tensor_copy(dst, ps_o)
                                    hold[("done", t2)] += 1
                                    if hold[("done", t2)] == 2:
                                        eng = (nc.sync if t2 % 2 == 0
                                               else nc.scalar)
                                        eng.dma_start(
                                            out_d[t2 * P:(t2 + 1) * P, :], ot
                                        )
                            ops.append(op)
                # group into (t2, n) units of MT c-ops each
                units = []
                for u in range(len(ops) // MT):
                    def unit(u=u):
                        for c in range(MT):
                            ops[u * MT + c]()
                    units.append(unit)
                return units

            # ---- block 0 rides the load window; scores interleave into the
            # projection grind so the exp stream starts early, including 6
            # early (0, t=1) steps whose PVs are stashed until the (0, t=0)
            # accumulators retire (PSUM acc-bank rotation is strictly
            # block-sequential) ----
            wo_first = deque(wo_units(range(0, 4), lambda t2, n: (pmm, "mm")))
            accs00 = []
            for qb in range(4):
                for op in proj_ops(0, "k", ns=[qb]):
                    op()
                for i in range(4 * qb, 4 * qb + 4):
                    emit_v(i)
                    pump_pv(4)
                if qb == 0:
                    for op in proj_ops(0, "q", ns=[0]):
                        op()
                for kc in range(4 * qb, 4 * qb + 2):
                    attn_scores(0, 0, kc, 0, 1, accs00)
                    pump_pv(4)
                if qb < 2:
                    for op in proj_ops(1, "k", ns=[qb]):
                        op()
                for kc in range(4 * qb + 2, 4 * qb + 4):
                    attn_scores(0, 0, kc, 0, 1, accs00)
                    pump_pv(4)
            for op in proj_ops(0, "q", ns=[1]):
                op()
                pump_pv(4)

            # ---- main attention blocks (filler BEFORE scores so chain
            # bias-adds precede the step's mask in the DVE queue) ----
            for hp in range(NHL // 2):
                h0, h1 = 2 * hp, 2 * hp + 1
                for t in (range(QT) if hp > 0 else [1]):
                    blk = 2 * hp + t
                    q = pending[blk]
                    accs = []
                    slots_left = KC
                    for kc in range(KC):
                        n_emit = -(-len(q) // slots_left)  # ceil
                        for _ in range(min(n_emit, len(q))):
                            q.popleft()()
                        slots_left -= 1
                        attn_scores(hp, t, kc, h0, h1, accs)
                        drains = (True if (hp, t) == (3, 1)
                                  else kc % 4 == 2)
                        if drains and (t == 1 or hp == 3):
                            if normb_pending:
                                normb_pending.popleft()()
                            if norm_pending:
                                norm_pending.popleft()()
                        if hp == 3 and t == 1 and kc >= 8 and wo_first:
                            wo_first.popleft()()
                        pump_pv(4)

            while wo_first:
                wo_first.popleft()()
            pump_pv(0)

            # ---- endgame: fused second-half output chains overlap the
            # final norm drains; PE stays dense so HAM stays warm ----
            ps_tail = {}

            def tail_head(t2):
                ps = psc.tile([P, 1024], f32, tag="sc", name=f"ps_o{t2}")
                ot = work.tile([P, 1024], bf16, tag="ot", name=f"ot{t2}",
                               bufs=3)
                ps_tail[t2] = (ps, ot)
                for c in range(3):
                    for n in range(2):
                        nc.tensor.matmul(
                            ps[:, n * 512:(n + 1) * 512],
                            lhsT=vals_sb[c][:, t2 * P:(t2 + 1) * P],
                            rhs=wo_sb[c][:, n * 512:(n + 1) * 512],
                            start=(c == 0), stop=False,
                        )

            def tail_fin(t2):
                ps, ot = ps_tail[t2]
                for n in range(2):
                    nc.tensor.matmul(
                        ps[:, n * 512:(n + 1) * 512],
                        lhsT=vals_sb[3][:, t2 * P:(t2 + 1) * P],
                        rhs=wo_sb[3][:, n * 512:(n + 1) * 512],
                        start=False, stop=True,
                    )
                nc.scalar.copy(ot[:, 0:512], ps[:, 0:512])
                nc.vector.tensor_copy(ot[:, 512:1024], ps[:, 512:1024])
                eng = nc.sync if t2 % 2 == 0 else nc.scalar
                eng.dma_start(out_d[t2 * P:(t2 + 1) * P, :], ot)

            while norm_pending:
                norm_pending.popleft()()
            tail_head(4)
            tail_head(5)
            # partial heads for t2=6 on the two pmm slots keep the PE dense
            # through the final norm drain (their c=3 finish comes after)
            ps6 = {}
            for n in range(2):
                ps6[n] = pmm.tile([P, 512], f32, tag="mm", name=f"ps_o6_{n}")
                for c in range(3):
                    nc.tensor.matmul(
                        ps6[n],
                        lhsT=vals_sb[c][:, 6 * P:7 * P],
                        rhs=wo_sb[c][:, n * 512:(n + 1) * 512],
                        start=(c == 0), stop=False,
                    )
            while normb_pending:
                normb_pending.popleft()()
            tail_fin(4)
            tail_fin(5)
            ot6 = work.tile([P, 1024], bf16, tag="ot", name="ot6", bufs=4)
            for n in range(2):
                nc.tensor.matmul(
                    ps6[n],
                    lhsT=vals_sb[3][:, 6 * P:7 * P],
                    rhs=wo_sb[3][:, n * 512:(n + 1) * 512],
                    start=False, stop=True,
                )
                if n == 0:
                    nc.scalar.copy(ot6[:, 0:512], ps6[0])
                else:
                    nc.vector.tensor_copy(ot6[:, 512:1024], ps6[1])
            nc.sync.dma_start(out_d[6 * P:7 * P, :], ot6)
            for u in wo_units(range(7, NQ // P), lambda t2, n: (pmm, "mm")):
                u()

    nc.compile()
    return nc


def _get_program():
    if "nc" not in _CACHE:
        _CACHE["nc"] = _build_program()
    return _CACHE["nc"]


def _per_core_inputs(x, y, mask, W_kv, b_kv, W_q, b_q, W_o):
    """Build the 8 per-core input maps (all staged row-contiguous)."""
    in_maps = []
    mask_f = mask.astype(np.float32)
    for c in range(8):
        b, g = c // 2, c % 2
        gh = np.arange(g * 8, g * 8 + 8)
        k_cols = (gh[:, None] * 2 * HD + np.arange(HD)[None, :]).ravel()
        v_cols = k_cols + HD
        q_cols = slice(g * 512, (g + 1) * 512)
        xb = np.ascontiguousarray(x[b].T).astype(BF)       # [D, NKV]
        yb = np.ascontiguousarray(y[b].T).astype(BF)       # [D, NQ]
        # quarter-major: row (qb*128+p), col (d*512+c) = xb[d*128+p, qb*512+c]
        xq = (xb.reshape(DC, P, 4, 512).transpose(2, 1, 0, 3)
              .reshape(4 * P, DC * 512))
        yh = (yb.reshape(DC, P, 2, 512).transpose(2, 1, 0, 3)
              .reshape(2 * P, DC * 512))

        def wstage(w):  # [1024, 512] -> [128, 8*512] row-contiguous
            return np.ascontiguousarray(
                w.reshape(DC, P, 512).transpose(1, 0, 2).reshape(P, DC * 512)
            ).astype(BF)

        wo_st = np.ascontiguousarray(
            W_o[q_cols, :].reshape(MT, P, D).transpose(1, 0, 2)
            .reshape(P, MT * D)
        ).astype(BF)
        in_maps.append({
            "xT": np.ascontiguousarray(xq),
            "yT": np.ascontiguousarray(yh),
            "maskT": np.ascontiguousarray(mask_f[b].T).astype(BF),
            "wk": wstage(W_kv[:, k_cols]),
            "wv": wstage(W_kv[:, v_cols]),
            "wq": wstage(W_q[:, q_cols]),
            "wo": wo_st,
            "bk": np.ascontiguousarray(
                b_kv[k_cols].astype(np.float32).reshape(MT, P).T),
            "bq": np.ascontiguousarray(
                b_q[np.arange(g * 512, (g + 1) * 512)]
                .astype(np.float32).reshape(MT, P).T),
        })
    return in_maps


def kernel(x, y, mask, W_kv, b_kv, W_q, b_q, W_o, b_o):
    from concourse import bass_utils

    x = np.asarray(x, np.float32)
    y = np.asarray(y, np.float32)
    mask = np.asarray(mask)
    W_kv = np.asarray(W_kv, np.float32)
    b_kv = np.asarray(b_kv, np.float32)
    W_q = np.asarray(W_q, np.float32)
    b_q = np.asarray(b_q, np.float32)
    W_o = np.asarray(W_o, np.float32)
    b_o = np.asarray(b_o, np.float32)

    nc = _get_program()
    in_maps = _per_core_inputs(x, y, mask, W_kv, b_kv, W_q, b_q, W_o)
    res = bass_utils.run_bass_kernel_spmd(nc, in_maps, core_ids=list(range(8)))

    # b_v folds into a constant row: attn rows sum to 1, so each head adds
    # b_v_h @ W_o_h to every output row; b_o adds on top.
    v_cols_all = (np.arange(H)[:, None] * 2 * HD + HD
                  + np.arange(HD)[None, :]).ravel()
    const_row = b_kv[v_cols_all].astype(np.float32) @ W_o + b_o

    out = np.empty((B, NQ, D), np.float32)
    for b in range(B):
        out[b] = (res.results[2 * b]["out"].astype(np.float32)
                  + res.results[2 * b + 1]["out"].astype(np.float32)
                  + const_row)
    return out


if __name__ == "__main__":
    import reference

    inputs = {k: np.asarray(v) for k, v in reference.setup_inputs().items()}
    got = kernel(**inputs)
    exp = np.asarray(reference.reference(**inputs))
    err = np.abs(got - exp)
    print("absmax rel err:", err.max() / np.abs(exp).max())


# revision 36
# speedup vs baseline: 1.0057x; 1.0057x over previous
"""Multi-head cross-attention Trainium2 Bass kernel, SPMD over 8 NeuronCores.

Sharding: core c handles batch b = c//2 and head group g = c%2 (8 of 16 heads).
Each core computes a partial output projection (its heads' W_o rows); the host
sums the two partials per batch element.

Device pipeline per core (all matmuls bf16 with fp32 PSUM accumulation):
  kT = (Wk^T x^T)          [512 hd, 2048 kseq]   (per-partition bias b_k)
  v  = (x Wv)              [2048 kseq, 8*65]     (65th col per head = ones)
  qT = (Wq^T y^T)          [512 hd, 1024 q]      (per-partition bias b_q)
  per (head-pair, q-tile, k-chunk):
      S^T[k, q|q'] = kT_h^T-chunk @ qT_h for both heads of the pair
        (K=64 row-tiled at partitions 0/64 -> the two matmuls run
         concurrently in the PE array; both write one 2-bank PSUM tile)
      em = exp(0.125 * S^T)  (one ACT op per pair; no row-max: |S|<=~25)
      em *= maskT            (one DVE mul per pair; mask chunk read once via
                              a step-0 broadcast dim)
      acc_h[65, q] += [v_h | 1]^T @ em_h   (row 64 = softmax denominator)
        -- the PV pair TRAILS its scores by 3 steps with projection filler
           emitted in between, so the PE queue (strict FIFO) never stalls
           on the exp->mask chain latency (~1.9us).
  normalize: vals_h = acc[0:64] * bcast(1/acc[64])  (PE outer-product bcast,
      reciprocal_approx_fast; deferred into later steps; softmax-drain
      copies run on DVE so ACT carries nothing but the exp stream)
  out_partial = vals^T-chunks @ Wo-rows  -> [1024 q, 1024 D] fp32
All host-staged inputs are laid out so every DMA source row is a multi-KB
contiguous run (1KB packets cap the DMA engines at ~200 GB/s; 8KB packets
saturate ~360 GB/s), and the first-needed tensors issue first across BOTH
HWDGE queues (sync + scalar) so the first kT chain starts ~12us in.
b_v and b_o fold into a host-side constant row (attn rows sum to 1).
"""

import sys
from collections import deque

import numpy as np
import ml_dtypes

if "/opt/trn_rl_repo" not in sys.path:
    sys.path.insert(0, "/opt/trn_rl_repo")

BF = ml_dtypes.bfloat16

B, NKV, NQ, D, H = 4, 2048, 1024, 1024, 16
HD = D // H          # 64
NHL = 8              # heads per core (local)
P = 128
DC = D // P          # 8 contraction chunks over model dim
KC = NKV // P        # 16 key-seq chunks
QT = NQ // 512       # 2 q tiles of 512 for attention
MT = 4               # hd-dim chunks of kT/qT (512/128)

_CACHE = {}


def _build_program():
    import concourse.bass as bass
    import concourse.mybir as mybir
    import concourse.tile as tile
    from concourse import bacc

    f32 = mybir.dt.float32
    bf16 = mybir.dt.bfloat16

    nc = bacc.Bacc(
        "TRN2", target_bir_lowering=False, debug=False, num_devices=8
    )

    # Host staging puts every tensor in the exact SBUF layout so each DMA
    # source ROW is one long contiguous run (-> 8KB packets, ~2x the DMA
    # throughput of the 1KB-segment layouts).
    xT_d = nc.dram_tensor("xT", [4 * P, DC * 512], bf16, kind="ExternalInput").ap()
    yT_d = nc.dram_tensor("yT", [2 * P, DC * 512], bf16, kind="ExternalInput").ap()
    maskT_d = nc.dram_tensor("maskT", [NKV, NQ], bf16, kind="ExternalInput").ap()
    wk_d = nc.dram_tensor("wk", [P, DC * 512], bf16, kind="ExternalInput").ap()
    wv_d = nc.dram_tensor("wv", [P, DC * 512], bf16, kind="ExternalInput").ap()
    wq_d = nc.dram_tensor("wq", [P, DC * 512], bf16, kind="ExternalInput").ap()
    wo_d = nc.dram_tensor("wo", [P, MT * D], bf16, kind="ExternalInput").ap()
    bk_d = nc.dram_tensor("bk", [P, MT], f32, kind="ExternalInput").ap()
    bq_d = nc.dram_tensor("bq", [P, MT], f32, kind="ExternalInput").ap()
    out_d = nc.dram_tensor("out", [NQ, D], bf16, kind="ExternalOutput").ap()

    Exp = mybir.ActivationFunctionType.Exp

    with tile.TileContext(nc) as tc:
        with (
            tc.tile_pool(name="persist", bufs=1) as persist,
            tc.tile_pool(name="work", bufs=3) as work,
            tc.tile_pool(name="empool", bufs=9) as empool,
            tc.tile_pool(name="pmm", bufs=2, space="PSUM") as pmm,
            tc.tile_pool(name="pacc", bufs=2, space="PSUM") as pacc,
            tc.tile_pool(name="psc", bufs=2, space="PSUM") as psc,
        ):
            wk_big = persist.tile([P, DC * 512], bf16, tag="wk", name="wk")
            wv_big = persist.tile([P, DC * 512], bf16, tag="wv", name="wv")
            wq_big = persist.tile([P, DC * 512], bf16, tag="wq", name="wq")
            wo_big = persist.tile([P, MT * D], bf16, tag="wo", name="wo")
            bk_big = persist.tile([P, MT], f32, tag="bk", name="bk")
            bq_big = persist.tile([P, MT], f32, tag="bq", name="bq")
            mT_big = persist.tile([P, KC * NQ], bf16, tag="mT", name="mT")
            xT_big = persist.tile([P, 4 * DC * 512], bf16, tag="xT", name="xT")
            yT_big = persist.tile([P, 2 * DC * 512], bf16, tag="yT", name="yT")

            wk_sb = [wk_big[:, d * 512:(d + 1) * 512] for d in range(DC)]
            wv_sb = [wv_big[:, d * 512:(d + 1) * 512] for d in range(DC)]
            wq_sb = [wq_big[:, d * 512:(d + 1) * 512] for d in range(DC)]
            wo_sb = [wo_big[:, c * D:(c + 1) * D] for c in range(MT)]
            bk_sb = [bk_big[:, m:m + 1] for m in range(MT)]
            bq_sb = [bq_big[:, m:m + 1] for m in range(MT)]
            maskT_sb = [mT_big[:, k * NQ:(k + 1) * NQ] for k in range(KC)]

            def xq(qb, d):   # x^T quarter qb (kseq cols 512qb..), d-chunk
                o = (qb * DC + d) * 512
                return xT_big[:, o:o + 512]

            def yq(nb, d):   # y^T half nb (q cols 512nb..), d-chunk
                o = (nb * DC + d) * 512
                return yT_big[:, o:o + 512]

            # ---- input DMAs: first-needed first, spread across the two
            # HWDGE queues (sync + scalar) so issue costs overlap ----
            def load_xt_q(qb, eng):
                eng.dma_start(
                    xT_big[:, qb * DC * 512:(qb + 1) * DC * 512],
                    xT_d[qb * P:(qb + 1) * P, :],
                )

            def load_yt_h(nb, eng):
                eng.dma_start(
                    yT_big[:, nb * DC * 512:(nb + 1) * DC * 512],
                    yT_d[nb * P:(nb + 1) * P, :],
                )

            def load_mask(k0, k1):
                dst = mT_big.rearrange("p (k c) -> p k c", k=KC)[:, k0:k1, :]
                src = maskT_d.rearrange("(k p) c -> p k c", k=KC, p=P)[
                    :, k0:k1, :]
                nc.sync.dma_start(dst, src)

            # first-needed tensors in halves so the first kT chain's d=0..3
            # matmuls unblock as soon as half the data lands
            HW2 = DC * 512 // 2
            HW4 = DC * 512 // 4
            # wk/xT quarters: the first kT chain's d=0,1 matmuls need only
            # 256KB per side, so the PE starts ~5us earlier on the ramping
            # DMA; later quarters land ahead of their d-chunks
            for pt in range(4):
                sl = slice(pt * HW4, (pt + 1) * HW4)
                nc.sync.dma_start(xT_big[:, sl], xT_d[0:P, sl])
                nc.scalar.dma_start(wk_big[:, sl], wk_d[:, sl])
            nc.sync.dma_start(wv_big[:, 0:HW2], wv_d[:, 0:HW2])
            nc.sync.dma_start(wv_big[:, HW2:2 * HW2], wv_d[:, HW2:2 * HW2])
            nc.scalar.dma_start(wq_big[:, 0:HW2], wq_d[:, 0:HW2])
            nc.scalar.dma_start(wq_big[:, HW2:2 * HW2], wq_d[:, HW2:2 * HW2])
            nc.sync.dma_start(yT_big[:, 0:HW2], yT_d[0:P, 0:HW2])
            nc.sync.dma_start(yT_big[:, HW2:2 * HW2], yT_d[0:P, HW2:2 * HW2])
            nc.scalar.dma_start(bk_big, bk_d)
            nc.scalar.dma_start(bq_big, bq_d)
            load_mask(0, 2)
            nc.scalar.dma_start(
                yT_big[:, DC * 512:2 * DC * 512], yT_d[P:2 * P, :])
            load_xt_q(1, nc.sync)
            load_mask(2, 4)
            load_mask(4, 8)
            load_xt_q(2, nc.sync)
            load_mask(8, 12)
            load_xt_q(3, nc.sync)
            load_mask(12, KC)
            nc.sync.dma_start(wo_big, wo_d)

            ones_sb = persist.tile([1, HD], bf16, tag="ones", name="ones")
            nc.gpsimd.memset(ones_sb, 1.0)

            kT_sb = [
                persist.tile([P, NKV], bf16, tag=f"kT{m}", name=f"kT{m}")
                for m in range(MT)
            ]
            qT_sb = [
                persist.tile([P, NQ], bf16, tag=f"qT{m}", name=f"qT{m}")
                for m in range(MT)
            ]
            v_sb = [
                persist.tile([P, NHL * 65], bf16, tag=f"v{i}", name=f"v{i}")
                for i in range(KC)
            ]
            for i in range(KC):
                nc.gpsimd.memset(
                    v_sb[i].rearrange("p (h c) -> p h c", c=65)[:, :, 64:65], 1.0
                )
            vals_sb = [
                persist.tile([P, NQ], bf16, tag=f"vals{c}", name=f"vals{c}")
                for c in range(MT)
            ]

            # ---- projection op queues (kT/qT chunk m as a list of closures,
            # one matmul each; the chain's last op appends the bias-add) ----
            def proj_ops(m, which, ns=None):
                w_sb, dst, bias, ncols = (
                    (wk_sb, kT_sb, bk_sb, NKV) if which == "k"
                    else (wq_sb, qT_sb, bq_sb, NQ)
                )
                ops = []
                hold = {}
                nlist = list(range(ncols // 512) if ns is None else ns)
                for n in nlist:
                    for d in range(DC):
                        def op(m=m, n=n, d=d, w_sb=w_sb, dst=dst, bias=bias,
                               which=which):
                            if d == 0:
                                hold[n] = pmm.tile(
                                    [P, 512], f32, tag="mm",
                                    name=f"pj{which}{m}_{n}"
                                )
                            nc.tensor.matmul(
                                hold[n],
                                lhsT=w_sb[d][:, m * P:(m + 1) * P],
                                rhs=(xq if which == "k" else yq)(n, d),
                                start=(d == 0),
                                stop=(d == DC - 1),
                            )
                            if d == DC - 1:
                                nc.vector.tensor_scalar_add(
                                    dst[m][:, n * 512:(n + 1) * 512],
                                    hold[n], bias[m]
                                )
                        ops.append(op)
                return ops

            def emit_v(i):
                ps_v = pmm.tile([P, 512], f32, tag="mm", name=f"ps_v{i}")
                for d in range(DC):
                    nc.tensor.matmul(
                        ps_v,
                        lhsT=xq(i // 4, d)[:, (i % 4) * P:(i % 4 + 1) * P],
                        rhs=wv_sb[d],
                        start=(d == 0),
                        stop=(d == DC - 1),
                    )
                v3 = v_sb[i].rearrange("p (h c) -> p h c", c=65)
                nc.vector.tensor_copy(
                    v3[:, :, 0:64], ps_v.rearrange("p (h c) -> p h c", c=64)
                )

            # ---- attention: scores/exp/mask emitted per step; the PV pair
            # trails by 2 steps (filler fills the gap) ----
            pv_backlog = deque()   # entries: (kc, closure)
            norm_pending = deque()

            def make_norm(hp, t, a, h, ut, s_f):
                po = a * HD
                qs = slice(t * 512, (t + 1) * 512)
                # the last block's norms drain in the endgame while both pmm
                # slots hold pre-emitted output-chain heads; their bcast
                # matmuls go to the freed pacc bank instead
                pool, ptag = ((pacc, "acc") if (hp, t) == (3, 1)
                              else (pmm, "mm"))

                def norm_op():
                    r_f = work.tile([1, 512], f32, tag="r", name=f"r{h}_{t}")
                    nc.vector.reciprocal_approx_fast(r_f, s_f)
                    r_b = work.tile([1, 512], bf16, tag="rb", name=f"rb{h}_{t}")
                    nc.vector.tensor_copy(r_b, r_f)
                    bps = pool.tile([HD, 512], f32, tag=ptag,
                                    name=f"bps{h}_{t}")
                    nc.tensor.matmul(
                        bps, lhsT=ones_sb, rhs=r_b, start=True, stop=True
                    )
                    nc.vector.tensor_mul(vals_sb[hp][po:po + HD, qs], ut, bps)
                return norm_op

            def drain_accs(hp, t, accs, h0, h1):
                for a, h in enumerate((h0, h1)):
                    acc = accs[a]
                    # free the PSUM accumulator quickly; all drain copies run
                    # on DVE so the ACT queue carries nothing but exps
                    ut = work.tile(
                        [HD, 512], bf16, tag="ut", name=f"ut{h}_{t}", bufs=4
                    )
                    nc.vector.tensor_copy(ut, acc[0:HD, :])
                    s_f = work.tile(
                        [1, 512], f32, tag="s", name=f"s{h}_{t}", bufs=5
                    )
                    nc.vector.tensor_copy(s_f, acc[64:65, :])
                    norm_pending.append(make_norm(hp, t, a, h, ut, s_f))

            def attn_scores(hp, t, kc, h0, h1, accs_box):
                qs = slice(t * 512, (t + 1) * 512)
                sp2 = psc.tile(
                    [P, 1024], f32, tag="sc", name=f"sp{hp}_{t}_{kc}"
                )
                for a in range(2):
                    po = a * HD
                    nc.tensor.matmul(
                        sp2[:, a * 512:(a + 1) * 512],
                        lhsT=kT_sb[hp][po:po + HD, kc * P:(kc + 1) * P],
                        rhs=qT_sb[hp][po:po + HD, qs],
                        start=True,
                        stop=True,
                    )
                em2 = empool.tile(
                    [P, 1024], bf16, tag="em", name=f"em{hp}_{t}_{kc}"
                )
                nc.scalar.activation(em2, sp2, Exp, scale=0.125)
                mb = (maskT_sb[kc][:, qs]
                      .rearrange("p (o q) -> p o q", o=1)
                      .broadcast_to([P, 2, 512]))
                em3 = em2.rearrange("p (o q) -> p o q", o=2)
                nc.vector.tensor_mul(em3, em3, mb)

                def pv(hp=hp, t=t, kc=kc):
                    if kc == 0:
                        accs_box[:] = [
                            pacc.tile([65, 512], f32, tag="acc",
                                      name=f"acc{h}_{t}")
                            for h in (h0, h1)
                        ]
                    for a, h in enumerate((h0, h1)):
                        nc.tensor.matmul(
                            accs_box[a],
                            lhsT=v_sb[kc][:, h * 65:(h + 1) * 65],
                            rhs=em2[:, a * 512:(a + 1) * 512],
                            start=(kc == 0),
                            stop=(kc == KC - 1),
                        )
                    if kc == KC - 1:
                        drain_accs(hp, t, accs_box, h0, h1)
                pv_backlog.append((kc, pv))

            # ---- deferred projections, drained as per-step filler ----
            pending = {
                1: deque(proj_ops(1, "q", ns=[0])
                         + proj_ops(1, "k", ns=[2, 3])
                         + proj_ops(1, "q", ns=[1])),
                2: deque(proj_ops(2, "k")),
                3: deque(proj_ops(2, "q")),
                4: deque(proj_ops(3, "k", ns=[0, 1, 2])),
                5: deque(proj_ops(3, "q")),
                6: deque(proj_ops(3, "k", ns=[3])),
                7: deque(),
            }

            def pump_pv(depth=2):
                while len(pv_backlog) > depth:
                    pv_backlog.popleft()[1]()

            def wo_units(t2s, pool_pick):
                # output-projection chains; ot tiles are [P, 1024] so the
                # stores are single fat DMAs; PSUM->SBUF copies alternate
                # ACT/DVE; store issues alternate sync/scalar queues
                ops = []
                hold = {}
                for t2 in t2s:
                    for n in range(2):
                        for c in range(MT):
                            def op(t2=t2, n=n, c=c):
                                if c == 0:
                                    pool, tag = pool_pick(t2, n)
                                    hold[(t2, n)] = pool.tile(
                                        [P, 512], f32, tag=tag,
                                        name=f"ps_o{t2}_{n}"
                                    )
                                    if ("ot", t2) not in hold:
                                        hold[("ot", t2)] = work.tile(
                                            [P, 1024], bf16, tag="ot",
                                            name=f"ot{t2}", bufs=3
                                        )
                                        hold[("done", t2)] = 0
                                ps_o = hold[(t2, n)]
                                nc.tensor.matmul(
                                    ps_o,
                                    lhsT=vals_sb[c][:, t2 * P:(t2 + 1) * P],
                                    rhs=wo_sb[c][:, n * 512:(n + 1) * 512],
                                    start=(c == 0),
                                    stop=(c == MT - 1),
                                )
                                if c == MT - 1:
                                    ot = hold[("ot", t2)]
                                    dst = ot[:, n * 512:(n + 1) * 512]
                                    if (t2 + n) % 2 == 0:
                                        nc.scalar.copy(dst, ps_o)
                                    else:
                                        nc.vector.tensor_copy(dst, ps_o)
                                    hold[("done", t2)] += 1
                                    if hold[("done", t2)] == 2:
                                        eng = (nc.sync if t2 % 2 == 0
                                               else nc.scalar)
                                        eng.dma_start(
                                            out_d[t2 * P:(t2 + 1) * P, :], ot
                                        )
                            ops.append(op)
                # group into (t2, n) units of MT c-ops each
                units = []
                for u in range(len(ops) // MT):
                    def unit(u=u):
                        for c in range(MT):
                            ops[u * MT + c]()
                    units.append(unit)
                return units

            # ---- block 0 rides the load window; scores interleave into the
            # projection grind so the exp stream starts early, including 6
            # early (0, t=1) steps whose PVs are stashed until the (0, t=0)
            # accumulators retire (PSUM acc-bank rotation is strictly
            # block-sequential) ----
            wo_first = deque(wo_units(range(0, 4), lambda t2, n: (pmm, "mm")))
            accs00 = []
            for qb in range(4):
                for op in proj_ops(0, "k", ns=[qb]):
                    op()
                for i in range(4 * qb, 4 * qb + 4):
                    emit_v(i)
                    pump_pv(4)
                if qb == 0:
                    for op in proj_ops(0, "q", ns=[0]):
                        op()
                for kc in range(4 * qb, 4 * qb + 2):
                    attn_scores(0, 0, kc, 0, 1, accs00)
                    pump_pv(4)
                if qb < 2:
                    for op in proj_ops(1, "k", ns=[qb]):
                        op()
                for kc in range(4 * qb + 2, 4 * qb + 4):
                    attn_scores(0, 0, kc, 0, 1, accs00)
                    pump_pv(4)
            for op in proj_ops(0, "q", ns=[1]):
                op()
                pump_pv(4)

            # ---- main attention blocks (filler BEFORE scores so chain
            # bias-adds precede the step's mask in the DVE queue) ----
            for hp in range(NHL // 2):
                h0, h1 = 2 * hp, 2 * hp + 1
                for t in (range(QT) if hp > 0 else [1]):
                    blk = 2 * hp + t
                    q = pending[blk]
                    accs = []
                    slots_left = KC
                    for kc in range(KC):
                        n_emit = -(-len(q) // slots_left)  # ceil
                        for _ in range(min(n_emit, len(q))):
                            q.popleft()()
                        slots_left -= 1
                        attn_scores(hp, t, kc, h0, h1, accs)
                        drains = (True if (hp, t) == (3, 1)
                                  else kc % 4 == 2)
                        if drains and (t == 1 or hp == 3) and norm_pending:
                            norm_pending.popleft()()
                        if hp == 3 and t == 1 and kc >= 8 and wo_first:
                            wo_first.popleft()()
                        pump_pv(4)

            while wo_first:
                wo_first.popleft()()
            pump_pv(0)

            # ---- endgame: fused second-half output chains overlap the
            # final norm drains; PE stays dense so HAM stays warm ----
            ps_tail = {}

            def tail_head(t2):
                ps = psc.tile([P, 1024], f32, tag="sc", name=f"ps_o{t2}")
                ot = work.tile([P, 1024], bf16, tag="ot", name=f"ot{t2}",
                               bufs=3)
                ps_tail[t2] = (ps, ot)
                for c in range(3):
                    for n in range(2):
                        nc.tensor.matmul(
                            ps[:, n * 512:(n + 1) * 512],
                            lhsT=vals_sb[c][:, t2 * P:(t2 + 1) * P],
                            rhs=wo_sb[c][:, n * 512:(n + 1) * 512],
                            start=(c == 0), stop=False,
                        )

            def tail_fin(t2):
                ps, ot = ps_tail[t2]
                for n in range(2):
                    nc.tensor.matmul(
                        ps[:, n * 512:(n + 1) * 512],
                        lhsT=vals_sb[3][:, t2 * P:(t2 + 1) * P],
                        rhs=wo_sb[3][:, n * 512:(n + 1) * 512],
                        start=False, stop=True,
                    )
                nc.scalar.copy(ot[:, 0:512], ps[:, 0:512])
                nc.vector.tensor_copy(ot[:, 512:1024], ps[:, 512:1024])
                eng = nc.sync if t2 % 2 == 0 else nc.scalar
                eng.dma_start(out_d[t2 * P:(t2 + 1) * P, :], ot)

            tail_head(4)
            tail_head(5)
            # partial heads for t2=6 on the two pmm slots keep the PE dense
            # through the final norm drain (their c=3 finish comes after)
            ps6 = {}
            for n in range(2):
                ps6[n] = pmm.tile([P, 512], f32, tag="mm", name=f"ps_o6_{n}")
                for c in range(3):
                    nc.tensor.matmul(
                        ps6[n],
                        lhsT=vals_sb[c][:, 6 * P:7 * P],
                        rhs=wo_sb[c][:, n * 512:(n + 1) * 512],
                        start=(c == 0), stop=False,
                    )
            while norm_pending:
                norm_pending.popleft()()
            tail_fin(4)
            tail_fin(5)
            ot6 = work.tile([P, 1024], bf16, tag="ot", name="ot6", bufs=4)
            for n in range(2):
                nc.tensor.matmul(
                    ps6[n],
                    lhsT=vals_sb[3][:, 6 * P:7 * P],
                    rhs=wo_sb[3][:, n * 512:(n + 1) * 512],
                    start=False, stop=True,
                )
                if n == 0:
                    nc.scalar.copy(ot6[:, 0:512], ps6[0])
                else:
                    nc.vector.tensor_copy(ot6[:, 512:1024], ps6[1])
            nc.sync.dma_start(out_d[6 * P:7 * P, :], ot6)
            for u in wo_units(range(7, NQ // P), lambda t2, n: (pmm, "mm")):
                u()

    nc.compile()
    return nc


def _get_program():
    if "nc" not in _CACHE:
        _CACHE["nc"] = _build_program()
    return _CACHE["nc"]


def _per_core_inputs(x, y, mask, W_kv, b_kv, W_q, b_q, W_o):
    """Build the 8 per-core input maps (all staged row-contiguous)."""
    in_maps = []
    mask_f = mask.astype(np.float32)
    for c in range(8):
        b, g = c // 2, c % 2
        gh = np.arange(g * 8, g * 8 + 8)
        k_cols = (gh[:, None] * 2 * HD + np.arange(HD)[None, :]).ravel()
        v_cols = k_cols + HD
        q_cols = slice(g * 512, (g + 1) * 512)
        xb = np.ascontiguousarray(x[b].T).astype(BF)       # [D, NKV]
        yb = np.ascontiguousarray(y[b].T).astype(BF)       # [D, NQ]
        # quarter-major: row (qb*128+p), col (d*512+c) = xb[d*128+p, qb*512+c]
        xq = (xb.reshape(DC, P, 4, 512).transpose(2, 1, 0, 3)
              .reshape(4 * P, DC * 512))
        yh = (yb.reshape(DC, P, 2, 512).transpose(2, 1, 0, 3)
              .reshape(2 * P, DC * 512))

        def wstage(w):  # [1024, 512] -> [128, 8*512] row-contiguous
            return np.ascontiguousarray(
                w.reshape(DC, P, 512).transpose(1, 0, 2).reshape(P, DC * 512)
            ).astype(BF)

        wo_st = np.ascontiguousarray(
            W_o[q_cols, :].reshape(MT, P, D).transpose(1, 0, 2)
            .reshape(P, MT * D)
        ).astype(BF)
        in_maps.append({
            "xT": np.ascontiguousarray(xq),
            "yT": np.ascontiguousarray(yh),
            "maskT": np.ascontiguousarray(mask_f[b].T).astype(BF),
            "wk": wstage(W_kv[:, k_cols]),
            "wv": wstage(W_kv[:, v_cols]),
            "wq": wstage(W_q[:, q_cols]),
            "wo": wo_st,
            "bk": np.ascontiguousarray(
                b_kv[k_cols].astype(np.float32).reshape(MT, P).T),
            "bq": np.ascontiguousarray(
                b_q[np.arange(g * 512, (g + 1) * 512)]
                .astype(np.float32).reshape(MT, P).T),
        })
    return in_maps


def kernel(x, y, mask, W_kv, b_kv, W_q, b_q, W_o, b_o):
    from concourse import bass_utils

    x = np.asarray(x, np.float32)
    y = np.asarray(y, np.float32)
    mask = np.asarray(mask)
    W_kv = np.asarray(W_kv, np.float32)
    b_kv = np.asarray(b_kv, np.float32)
    W_q = np.asarray(W_q, np.float32)
    b_q = np.asarray(b_q, np.float32)
    W_o = np.asarray(W_o, np.float32)
    b_o = np.asarray(b_o, np.float32)

    nc = _get_program()
    in_maps = _per_core_inputs(x, y, mask, W_kv, b_kv, W_q, b_q, W_o)
    res = bass_utils.run_bass_kernel_spmd(nc, in_maps, core_ids=list(range(8)))

    # b_v folds into a constant row: attn rows sum to 1, so each head adds
    # b_v_h @ W_o_h to every output row; b_o adds on top.
    v_cols_all = (np.arange(H)[:, None] * 2 * HD + HD
                  + np.arange(HD)[None, :]).ravel()
    const_row = b_kv[v_cols_all].astype(np.float32) @ W_o + b_o

    out = np.empty((B, NQ, D), np.float32)
    for b in range(B):
        out[b] = (res.results[2 * b]["out"].astype(np.float32)
                  + res.results[2 * b + 1]["out"].astype(np.float32)
                  + const_row)
    return out


if __name__ == "__main__":
    import reference

    inputs = {k: np.asarray(v) for k, v in reference.setup_inputs().items()}
    got = kernel(**inputs)
    exp = np.asarray(reference.reference(**inputs))
    err = np.abs(got - exp)
    print("absmax rel err:", err.max() / np.abs(exp).max())


# revision 38
# speedup vs baseline: 1.0075x; 1.0017x over previous
"""Multi-head cross-attention Trainium2 Bass kernel, SPMD over 8 NeuronCores.

Sharding: core c handles batch b = c//2 and head group g = c%2 (8 of 16 heads).
Each core computes a partial output projection (its heads' W_o rows); the host
sums the two partials per batch element.

Device pipeline per core (all matmuls bf16 with fp32 PSUM accumulation):
  kT = (Wk^T x^T)          [512 hd, 2048 kseq]   (per-partition bias b_k)
  v  = (x Wv)              [2048 kseq, 8*65]     (65th col per head = ones)
  qT = (Wq^T y^T)          [512 hd, 1024 q]      (per-partition bias b_q)
  per (head-pair, q-tile, k-chunk):
      S^T[k, q|q'] = kT_h^T-chunk @ qT_h for both heads of the pair
        (K=64 row-tiled at partitions 0/64 -> the two matmuls run
         concurrently in the PE array; both write one 2-bank PSUM tile)
      em = exp(0.125 * S^T)  (one ACT op per pair; no row-max: |S|<=~25)
      em *= maskT            (one DVE mul per pair; mask chunk read once via
                              a step-0 broadcast dim)
      acc_h[65, q] += [v_h | 1]^T @ em_h   (row 64 = softmax denominator)
        -- the PV pair TRAILS its scores by 3 steps with projection filler
           emitted in between, so the PE queue (strict FIFO) never stalls
           on the exp->mask chain latency (~1.9us).
  normalize: vals_h = acc[0:64] * bcast(1/acc[64])  (PE outer-product bcast,
      reciprocal_approx_fast; deferred into later steps; softmax-drain
      copies run on DVE so ACT carries nothing but the exp stream)
  out_partial = vals^T-chunks @ Wo-rows  -> [1024 q, 1024 D] fp32
All host-staged inputs are laid out so every DMA source row is a multi-KB
contiguous run (1KB packets cap the DMA engines at ~200 GB/s; 8KB packets
saturate ~360 GB/s), and the first-needed tensors issue first across BOTH
HWDGE queues (sync + scalar) so the first kT chain starts ~12us in.
b_v and b_o fold into a host-side constant row (attn rows sum to 1).
"""

import sys
from collections import deque

import numpy as np
import ml_dtypes

if "/opt/trn_rl_repo" not in sys.path:
    sys.path.insert(0, "/opt/trn_rl_repo")

BF = ml_dtypes.bfloat16

B, NKV, NQ, D, H = 4, 2048, 1024, 1024, 16
HD = D // H          # 64
NHL = 8              # heads per core (local)
P = 128
DC = D // P          # 8 contraction chunks over model dim
KC = NKV // P        # 16 key-seq chunks
QT = NQ // 512       # 2 q tiles of 512 for attention
MT = 4               # hd-dim chunks of kT/qT (512/128)

_CACHE = {}


def _build_program():
    import concourse.bass as bass
    import concourse.mybir as mybir
    import concourse.tile as tile
    from concourse import bacc

    f32 = mybir.dt.float32
    bf16 = mybir.dt.bfloat16

    nc = bacc.Bacc(
        "TRN2", target_bir_lowering=False, debug=False, num_devices=8
    )

    # Host staging puts every tensor in the exact SBUF layout so each DMA
    # source ROW is one long contiguous run (-> 8KB packets, ~2x the DMA
    # throughput of the 1KB-segment layouts).
    xT_d = nc.dram_tensor("xT", [4 * P, DC * 512], bf16, kind="ExternalInput").ap()
    yT_d = nc.dram_tensor("yT", [2 * P, DC * 512], bf16, kind="ExternalInput").ap()
    maskT_d = nc.dram_tensor("maskT", [NKV, NQ], bf16, kind="ExternalInput").ap()
    wk_d = nc.dram_tensor("wk", [P, DC * 512], bf16, kind="ExternalInput").ap()
    wv_d = nc.dram_tensor("wv", [P, DC * 512], bf16, kind="ExternalInput").ap()
    wq_d = nc.dram_tensor("wq", [P, DC * 512], bf16, kind="ExternalInput").ap()
    wo_d = nc.dram_tensor("wo", [P, MT * D], bf16, kind="ExternalInput").ap()
    bk_d = nc.dram_tensor("bk", [P, MT], f32, kind="ExternalInput").ap()
    bq_d = nc.dram_tensor("bq", [P, MT], f32, kind="ExternalInput").ap()
    out_d = nc.dram_tensor("out", [NQ, D], bf16, kind="ExternalOutput").ap()

    Exp = mybir.ActivationFunctionType.Exp

    with tile.TileContext(nc) as tc:
        with (
            tc.tile_pool(name="persist", bufs=1) as persist,
            tc.tile_pool(name="work", bufs=3) as work,
            tc.tile_pool(name="empool", bufs=9) as empool,
            tc.tile_pool(name="pmm", bufs=2, space="PSUM") as pmm,
            tc.tile_pool(name="pacc", bufs=2, space="PSUM") as pacc,
            tc.tile_pool(name="psc", bufs=2, space="PSUM") as psc,
        ):
            wk_big = persist.tile([P, DC * 512], bf16, tag="wk", name="wk")
            wv_big = persist.tile([P, DC * 512], bf16, tag="wv", name="wv")
            wq_big = persist.tile([P, DC * 512], bf16, tag="wq", name="wq")
            wo_big = persist.tile([P, MT * D], bf16, tag="wo", name="wo")
            bk_big = persist.tile([P, MT], f32, tag="bk", name="bk")
            bq_big = persist.tile([P, MT], f32, tag="bq", name="bq")
            mT_big = persist.tile([P, KC * NQ], bf16, tag="mT", name="mT")
            xT_big = persist.tile([P, 4 * DC * 512], bf16, tag="xT", name="xT")
            yT_big = persist.tile([P, 2 * DC * 512], bf16, tag="yT", name="yT")

            wk_sb = [wk_big[:, d * 512:(d + 1) * 512] for d in range(DC)]
            wv_sb = [wv_big[:, d * 512:(d + 1) * 512] for d in range(DC)]
            wq_sb = [wq_big[:, d * 512:(d + 1) * 512] for d in range(DC)]
            wo_sb = [wo_big[:, c * D:(c + 1) * D] for c in range(MT)]
            bk_sb = [bk_big[:, m:m + 1] for m in range(MT)]
            bq_sb = [bq_big[:, m:m + 1] for m in range(MT)]
            maskT_sb = [mT_big[:, k * NQ:(k + 1) * NQ] for k in range(KC)]

            def xq(qb, d):   # x^T quarter qb (kseq cols 512qb..), d-chunk
                o = (qb * DC + d) * 512
                return xT_big[:, o:o + 512]

            def yq(nb, d):   # y^T half nb (q cols 512nb..), d-chunk
                o = (nb * DC + d) * 512
                return yT_big[:, o:o + 512]

            # ---- input DMAs: first-needed first, spread across the two
            # HWDGE queues (sync + scalar) so issue costs overlap ----
            def load_xt_q(qb, eng):
                eng.dma_start(
                    xT_big[:, qb * DC * 512:(qb + 1) * DC * 512],
                    xT_d[qb * P:(qb + 1) * P, :],
                )

            def load_yt_h(nb, eng):
                eng.dma_start(
                    yT_big[:, nb * DC * 512:(nb + 1) * DC * 512],
                    yT_d[nb * P:(nb + 1) * P, :],
                )

            def load_mask(k0, k1):
                dst = mT_big.rearrange("p (k c) -> p k c", k=KC)[:, k0:k1, :]
                src = maskT_d.rearrange("(k p) c -> p k c", k=KC, p=P)[
                    :, k0:k1, :]
                nc.sync.dma_start(dst, src)

            # first-needed tensors in halves so the first kT chain's d=0..3
            # matmuls unblock as soon as half the data lands
            HW2 = DC * 512 // 2
            nc.sync.dma_start(xT_big[:, 0:HW2], xT_d[0:P, 0:HW2])
            nc.scalar.dma_start(wk_big[:, 0:HW2], wk_d[:, 0:HW2])
            nc.sync.dma_start(xT_big[:, HW2:2 * HW2], xT_d[0:P, HW2:2 * HW2])
            nc.scalar.dma_start(wk_big[:, HW2:2 * HW2], wk_d[:, HW2:2 * HW2])
            nc.sync.dma_start(wv_big[:, 0:HW2], wv_d[:, 0:HW2])
            nc.sync.dma_start(wv_big[:, HW2:2 * HW2], wv_d[:, HW2:2 * HW2])
            nc.scalar.dma_start(wq_big[:, 0:HW2], wq_d[:, 0:HW2])
            nc.scalar.dma_start(wq_big[:, HW2:2 * HW2], wq_d[:, HW2:2 * HW2])
            load_yt_h(0, nc.sync)
            nc.scalar.dma_start(bk_big, bk_d)
            nc.scalar.dma_start(bq_big, bq_d)
            load_mask(0, 2)
            nc.scalar.dma_start(
                yT_big[:, DC * 512:2 * DC * 512], yT_d[P:2 * P, :])
            load_xt_q(1, nc.sync)
            load_mask(2, 4)
            load_mask(4, 8)
            load_xt_q(2, nc.sync)
            load_mask(8, 12)
            load_xt_q(3, nc.sync)
            load_mask(12, KC)
            nc.sync.dma_start(wo_big, wo_d)

            ones_sb = persist.tile([1, HD], bf16, tag="ones", name="ones")
            nc.gpsimd.memset(ones_sb, 1.0)

            kT_sb = [
                persist.tile([P, NKV], bf16, tag=f"kT{m}", name=f"kT{m}")
                for m in range(MT)
            ]
            qT_sb = [
                persist.tile([P, NQ], bf16, tag=f"qT{m}", name=f"qT{m}")
                for m in range(MT)
            ]
            v_sb = [
                persist.tile([P, NHL * 65], bf16, tag=f"v{i}", name=f"v{i}")
                for i in range(KC)
            ]
            for i in range(KC):
                nc.gpsimd.memset(
                    v_sb[i].rearrange("p (h c) -> p h c", c=65)[:, :, 64:65], 1.0
                )
            vals_sb = [
                persist.tile([P, NQ], bf16, tag=f"vals{c}", name=f"vals{c}")
                for c in range(MT)
            ]

            # ---- projection op queues (kT/qT chunk m as a list of closures,
            # one matmul each; the chain's last op appends the bias-add) ----
            def proj_ops(m, which, ns=None):
                w_sb, dst, bias, ncols = (
                    (wk_sb, kT_sb, bk_sb, NKV) if which == "k"
                    else (wq_sb, qT_sb, bq_sb, NQ)
                )
                ops = []
                hold = {}
                nlist = list(range(ncols // 512) if ns is None else ns)
                for n in nlist:
                    for d in range(DC):
                        def op(m=m, n=n, d=d, w_sb=w_sb, dst=dst, bias=bias,
                               which=which):
                            if d == 0:
                                hold[n] = pmm.tile(
                                    [P, 512], f32, tag="mm",
                                    name=f"pj{which}{m}_{n}"
                                )
                            nc.tensor.matmul(
                                hold[n],
                                lhsT=w_sb[d][:, m * P:(m + 1) * P],
                                rhs=(xq if which == "k" else yq)(n, d),
                                start=(d == 0),
                                stop=(d == DC - 1),
                            )
                            if d == DC - 1:
                                nc.vector.tensor_scalar_add(
                                    dst[m][:, n * 512:(n + 1) * 512],
                                    hold[n], bias[m]
                                )
                        ops.append(op)
                return ops

            def emit_v(i):
                ps_v = pmm.tile([P, 512], f32, tag="mm", name=f"ps_v{i}")
                for d in range(DC):
                    nc.tensor.matmul(
                        ps_v,
                        lhsT=xq(i // 4, d)[:, (i % 4) * P:(i % 4 + 1) * P],
                        rhs=wv_sb[d],
                        start=(d == 0),
                        stop=(d == DC - 1),
                    )
                v3 = v_sb[i].rearrange("p (h c) -> p h c", c=65)
                # v drains run on ACT: they all fall in block0 where the exp
                # stream is data-paced (ACT idle), and off DVE they stop
                # delaying the mask multiplies that gate the PV pairs
                nc.scalar.copy(
                    v3[:, :, 0:64], ps_v.rearrange("p (h c) -> p h c", c=64)
                )

            # ---- attention: scores/exp/mask emitted per step; the PV pair
            # trails by 2 steps (filler fills the gap) ----
            pv_backlog = deque()   # entries: (kc, closure)
            norm_pending = deque()

            def make_norm(hp, t, a, h, ut, s_f):
                po = a * HD
                qs = slice(t * 512, (t + 1) * 512)
                # the last block's norms drain in the endgame while both pmm
                # slots hold pre-emitted output-chain heads; their bcast
                # matmuls go to the freed pacc bank instead
                pool, ptag = ((pacc, "acc") if (hp, t) == (3, 1)
                              else (pmm, "mm"))

                def norm_op():
                    r_f = work.tile([1, 512], f32, tag="r", name=f"r{h}_{t}")
                    nc.vector.reciprocal_approx_fast(r_f, s_f)
                    r_b = work.tile([1, 512], bf16, tag="rb", name=f"rb{h}_{t}")
                    nc.vector.tensor_copy(r_b, r_f)
                    bps = pool.tile([HD, 512], f32, tag=ptag,
                                    name=f"bps{h}_{t}")
                    nc.tensor.matmul(
                        bps, lhsT=ones_sb, rhs=r_b, start=True, stop=True
                    )
                    nc.vector.tensor_mul(vals_sb[hp][po:po + HD, qs], ut, bps)
                return norm_op

            def drain_accs(hp, t, accs, h0, h1):
                for a, h in enumerate((h0, h1)):
                    acc = accs[a]
                    # free the PSUM accumulator quickly; all drain copies run
                    # on DVE so the ACT queue carries nothing but exps
                    ut = work.tile(
                        [HD, 512], bf16, tag="ut", name=f"ut{h}_{t}", bufs=4
                    )
                    nc.vector.tensor_copy(ut, acc[0:HD, :])
                    s_f = work.tile(
                        [1, 512], f32, tag="s", name=f"s{h}_{t}", bufs=5
                    )
                    nc.vector.tensor_copy(s_f, acc[64:65, :])
                    norm_pending.append(make_norm(hp, t, a, h, ut, s_f))

            def attn_scores(hp, t, kc, h0, h1, accs_box):
                qs = slice(t * 512, (t + 1) * 512)
                sp2 = psc.tile(
                    [P, 1024], f32, tag="sc", name=f"sp{hp}_{t}_{kc}"
                )
                for a in range(2):
                    po = a * HD
                    nc.tensor.matmul(
                        sp2[:, a * 512:(a + 1) * 512],
                        lhsT=kT_sb[hp][po:po + HD, kc * P:(kc + 1) * P],
                        rhs=qT_sb[hp][po:po + HD, qs],
                        start=True,
                        stop=True,
                    )
                em2 = empool.tile(
                    [P, 1024], bf16, tag="em", name=f"em{hp}_{t}_{kc}"
                )
                nc.scalar.activation(em2, sp2, Exp, scale=0.125)
                mb = (maskT_sb[kc][:, qs]
                      .rearrange("p (o q) -> p o q", o=1)
                      .broadcast_to([P, 2, 512]))
                em3 = em2.rearrange("p (o q) -> p o q", o=2)
                nc.vector.tensor_mul(em3, em3, mb)

                def pv(hp=hp, t=t, kc=kc):
                    if kc == 0:
                        accs_box[:] = [
                            pacc.tile([65, 512], f32, tag="acc",
                                      name=f"acc{h}_{t}")
                            for h in (h0, h1)
                        ]
                    for a, h in enumerate((h0, h1)):
                        nc.tensor.matmul(
                            accs_box[a],
                            lhsT=v_sb[kc][:, h * 65:(h + 1) * 65],
                            rhs=em2[:, a * 512:(a + 1) * 512],
                            start=(kc == 0),
                            stop=(kc == KC - 1),
                        )
                    if kc == KC - 1:
                        drain_accs(hp, t, accs_box, h0, h1)
                pv_backlog.append((kc, pv))

            # ---- deferred projections, drained as per-step filler ----
            pending = {
                1: deque(proj_ops(1, "q", ns=[0])
                         + proj_ops(1, "k", ns=[2, 3])
                         + proj_ops(1, "q", ns=[1])),
                2: deque(proj_ops(2, "k")),
                3: deque(proj_ops(2, "q")),
                4: deque(proj_ops(3, "k", ns=[0, 1, 2])),
                5: deque(proj_ops(3, "q")),
                6: deque(proj_ops(3, "k", ns=[3])),
                7: deque(),
            }

            def pump_pv(depth=2):
                while len(pv_backlog) > depth:
                    pv_backlog.popleft()[1]()

            def wo_units(t2s, pool_pick):
                # output-projection chains; ot tiles are [P, 1024] so the
                # stores are single fat DMAs; PSUM->SBUF copies alternate
                # ACT/DVE; store issues alternate sync/scalar queues
                ops = []
                hold = {}
                for t2 in t2s:
                    for n in range(2):
                        for c in range(MT):
                            def op(t2=t2, n=n, c=c):
                                if c == 0:
                                    pool, tag = pool_pick(t2, n)
                                    hold[(t2, n)] = pool.tile(
                                        [P, 512], f32, tag=tag,
                                        name=f"ps_o{t2}_{n}"
                                    )
                                    if ("ot", t2) not in hold:
                                        hold[("ot", t2)] = work.tile(
                                            [P, 1024], bf16, tag="ot",
                                            name=f"ot{t2}", bufs=3
                                        )
                                        hold[("done", t2)] = 0
                                ps_o = hold[(t2, n)]
                                nc.tensor.matmul(
                                    ps_o,
                                    lhsT=vals_sb[c][:, t2 * P:(t2 + 1) * P],
                                    rhs=wo_sb[c][:, n * 512:(n + 1) * 512],
                                    start=(c == 0),
                                    stop=(c == MT - 1),
                                )
                                if c == MT - 1:
                                    ot = hold[("ot", t2)]
                                    dst = ot[:, n * 512:(n + 1) * 512]
                                    if (t2 + n) % 2 == 0:
                                        nc.scalar.copy(dst, ps_o)
                                    else:
                                        nc.vector.tensor_copy(dst, ps_o)
                                    hold[("done", t2)] += 1
                                    if hold[("done", t2)] == 2:
                                        eng = (nc.sync if t2 % 2 == 0
                                               else nc.scalar)
                                        eng.dma_start(
                                            out_d[t2 * P:(t2 + 1) * P, :], ot
                                        )
                            ops.append(op)
                # group into (t2, n) units of MT c-ops each
                units = []
                for u in range(len(ops) // MT):
                    def unit(u=u):
                        for c in range(MT):
                            ops[u * MT + c]()
                    units.append(unit)
                return units

            # ---- block 0 rides the load window; scores interleave into the
            # projection grind so the exp stream starts early, including 6
            # early (0, t=1) steps whose PVs are stashed until the (0, t=0)
            # accumulators retire (PSUM acc-bank rotation is strictly
            # block-sequential) ----
            wo_first = deque(wo_units(range(0, 4), lambda t2, n: (pmm, "mm")))
            accs00 = []
            for qb in range(4):
                for op in proj_ops(0, "k", ns=[qb]):
                    op()
                for i in range(4 * qb, 4 * qb + 4):
                    emit_v(i)
                    pump_pv(4)
                if qb == 0:
                    for op in proj_ops(0, "q", ns=[0]):
                        op()
                for kc in range(4 * qb, 4 * qb + 2):
                    attn_scores(0, 0, kc, 0, 1, accs00)
                    pump_pv(4)
                if qb < 2:
                    for op in proj_ops(1, "k", ns=[qb]):
                        op()
                for kc in range(4 * qb + 2, 4 * qb + 4):
                    attn_scores(0, 0, kc, 0, 1, accs00)
                    pump_pv(4)
            for op in proj_ops(0, "q", ns=[1]):
                op()
                pump_pv(4)

            # ---- main attention blocks (filler BEFORE scores so chain
            # bias-adds precede the step's mask in the DVE queue) ----
            for hp in range(NHL // 2):
                h0, h1 = 2 * hp, 2 * hp + 1
                for t in (range(QT) if hp > 0 else [1]):
                    blk = 2 * hp + t
                    q = pending[blk]
                    accs = []
                    slots_left = KC
                    for kc in range(KC):
                        n_emit = -(-len(q) // slots_left)  # ceil
                        for _ in range(min(n_emit, len(q))):
                            q.popleft()()
                        slots_left -= 1
                        attn_scores(hp, t, kc, h0, h1, accs)
                        drains = (True if (hp, t) == (3, 1)
                                  else kc % 4 == 2)
                        if drains and (t == 1 or hp == 3) and norm_pending:
                            norm_pending.popleft()()
                        if hp == 3 and t == 1 and kc >= 8 and wo_first:
                            wo_first.popleft()()
                        pump_pv(4)

            while wo_first:
                wo_first.popleft()()
            pump_pv(0)

            # ---- endgame: fused second-half output chains overlap the
            # final norm drains; PE stays dense so HAM stays warm ----
            ps_tail = {}

            def tail_head(t2):
                ps = psc.tile([P, 1024], f32, tag="sc", name=f"ps_o{t2}")
                ot = work.tile([P, 1024], bf16, tag="ot", name=f"ot{t2}",
                               bufs=3)
                ps_tail[t2] = (ps, ot)
                for c in range(3):
                    for n in range(2):
                        nc.tensor.matmul(
                            ps[:, n * 512:(n + 1) * 512],
                            lhsT=vals_sb[c][:, t2 * P:(t2 + 1) * P],
                            rhs=wo_sb[c][:, n * 512:(n + 1) * 512],
                            start=(c == 0), stop=False,
                        )

            def tail_fin(t2):
                ps, ot = ps_tail[t2]
                for n in range(2):
                    nc.tensor.matmul(
                        ps[:, n * 512:(n + 1) * 512],
                        lhsT=vals_sb[3][:, t2 * P:(t2 + 1) * P],
                        rhs=wo_sb[3][:, n * 512:(n + 1) * 512],
                        start=False, stop=True,
                    )
                nc.scalar.copy(ot[:, 0:512], ps[:, 0:512])
                nc.vector.tensor_copy(ot[:, 512:1024], ps[:, 512:1024])
                eng = nc.sync if t2 % 2 == 0 else nc.scalar
                eng.dma_start(out_d[t2 * P:(t2 + 1) * P, :], ot)

            tail_head(4)
            tail_head(5)
            # partial heads for t2=6 on the two pmm slots keep the PE dense
            # through the final norm drain (their c=3 finish comes after)
            ps6 = {}
            for n in range(2):
                ps6[n] = pmm.tile([P, 512], f32, tag="mm", name=f"ps_o6_{n}")
                for c in range(3):
                    nc.tensor.matmul(
                        ps6[n],
                        lhsT=vals_sb[c][:, 6 * P:7 * P],
                        rhs=wo_sb[c][:, n * 512:(n + 1) * 512],
                        start=(c == 0), stop=False,
                    )
            while norm_pending:
                norm_pending.popleft()()
            tail_fin(4)
            tail_fin(5)
            ot6 = work.tile([P, 1024], bf16, tag="ot", name="ot6", bufs=4)
            for n in range(2):
                nc.tensor.matmul(
                    ps6[n],
                    lhsT=vals_sb[3][:, 6 * P:7 * P],
                    rhs=wo_sb[3][:, n * 512:(n + 1) * 512],
                    start=False, stop=True,
                )
                if n == 0:
                    nc.scalar.copy(ot6[:, 0:512], ps6[0])
                else:
                    nc.vector.tensor_copy(ot6[:, 512:1024], ps6[1])
            nc.sync.dma_start(out_d[6 * P:7 * P, :], ot6)
            for u in wo_units(range(7, NQ // P), lambda t2, n: (pmm, "mm")):
                u()

    nc.compile()
    return nc


def _get_program():
    if "nc" not in _CACHE:
        _CACHE["nc"] = _build_program()
    return _CACHE["nc"]


def _per_core_inputs(x, y, mask, W_kv, b_kv, W_q, b_q, W_o):
    """Build the 8 per-core input maps (all staged row-contiguous)."""
    in_maps = []
    mask_f = mask.astype(np.float32)
    for c in range(8):
        b, g = c // 2, c % 2
        gh = np.arange(g * 8, g * 8 + 8)
        k_cols = (gh[:, None] * 2 * HD + np.arange(HD)[None, :]).ravel()
        v_cols = k_cols + HD
        q_cols = slice(g * 512, (g + 1) * 512)
        xb = np.ascontiguousarray(x[b].T).astype(BF)       # [D, NKV]
        yb = np.ascontiguousarray(y[b].T).astype(BF)       # [D, NQ]
        # quarter-major: row (qb*128+p), col (d*512+c) = xb[d*128+p, qb*512+c]
        xq = (xb.reshape(DC, P, 4, 512).transpose(2, 1, 0, 3)
              .reshape(4 * P, DC * 512))
        yh = (yb.reshape(DC, P, 2, 512).transpose(2, 1, 0, 3)
              .reshape(2 * P, DC * 512))

        def wstage(w):  # [1024, 512] -> [128, 8*512] row-contiguous
            return np.ascontiguousarray(
                w.reshape(DC, P, 512).transpose(1, 0, 2).reshape(P, DC * 512)
            ).astype(BF)

        wo_st = np.ascontiguousarray(
            W_o[q_cols, :].reshape(MT, P, D).transpose(1, 0, 2)
            .reshape(P, MT * D)
        ).astype(BF)
        in_maps.append({
            "xT": np.ascontiguousarray(xq),
            "yT": np.ascontiguousarray(yh),
            "maskT": np.ascontiguousarray(mask_f[b].T).astype(BF),
            "wk": wstage(W_kv[:, k_cols]),
            "wv": wstage(W_kv[:, v_cols]),
            "wq": wstage(W_q[:, q_cols]),
            "wo": wo_st,
            "bk": np.ascontiguousarray(
                b_kv[k_cols].astype(np.float32).reshape(MT, P).T),
            "bq": np.ascontiguousarray(
                b_q[np.arange(g * 512, (g + 1) * 512)]
                .astype(np.float32).reshape(MT, P).T),
        })
    return in_maps


def kernel(x, y, mask, W_kv, b_kv, W_q, b_q, W_o, b_o):
    from concourse import bass_utils

    x = np.asarray(x, np.float32)
    y = np.asarray(y, np.float32)
    mask = np.asarray(mask)
    W_kv = np.asarray(W_kv, np.float32)
    b_kv = np.asarray(b_kv, np.float32)
    W_q = np.asarray(W_q, np.float32)
    b_q = np.asarray(b_q, np.float32)
    W_o = np.asarray(W_o, np.float32)
    b_o = np.asarray(b_o, np.float32)

    nc = _get_program()
    in_maps = _per_core_inputs(x, y, mask, W_kv, b_kv, W_q, b_q, W_o)
    res = bass_utils.run_bass_kernel_spmd(nc, in_maps, core_ids=list(range(8)))

    # b_v folds into a constant row: attn rows sum to 1, so each head adds
    # b_v_h @ W_o_h to every output row; b_o adds on top.
    v_cols_all = (np.arange(H)[:, None] * 2 * HD + HD
                  + np.arange(HD)[None, :]).ravel()
    const_row = b_kv[v_cols_all].astype(np.float32) @ W_o + b_o

    out = np.empty((B, NQ, D), np.float32)
    for b in range(B):
        out[b] = (res.results[2 * b]["out"].astype(np.float32)
                  + res.results[2 * b + 1]["out"].astype(np.float32)
                  + const_row)
    return out


if __name__ == "__main__":
    import reference

    inputs = {k: np.asarray(v) for k, v in reference.setup_inputs().items()}
    got = kernel(**inputs)
    exp = np.asarray(reference.reference(**inputs))
    err = np.abs(got - exp)
    print("absmax rel err:", err.max() / np.abs(exp).max())


# revision 40
# speedup vs baseline: 1.0181x; 1.0106x over previous
"""Multi-head cross-attention Trainium2 Bass kernel, SPMD over 8 NeuronCores.

Sharding: core c handles batch b = c//2 and head group g = c%2 (8 of 16 heads).
Each core computes a partial output projection (its heads' W_o rows); the host
sums the two partials per batch element.

Device pipeline per core (all matmuls bf16 with fp32 PSUM accumulation):
  kT = (Wk^T x^T)          [512 hd, 2048 kseq]   (per-partition bias b_k)
  v  = (x Wv)              [2048 kseq, 8*65]     (65th col per head = ones)
  qT = (Wq^T y^T)          [512 hd, 1024 q]      (per-partition bias b_q)
  per (head-pair, q-tile, k-chunk):
      S^T[k, q|q'] = kT_h^T-chunk @ qT_h for both heads of the pair
        (K=64 row-tiled at partitions 0/64 -> the two matmuls run
         concurrently in the PE array; both write one 2-bank PSUM tile)
      em = exp(0.125 * S^T)  (one ACT op per pair; no row-max: |S|<=~25)
      em *= maskT            (one DVE mul per pair; mask chunk read once via
                              a step-0 broadcast dim)
      acc_h[65, q] += [v_h | 1]^T @ em_h   (row 64 = softmax denominator)
        -- the PV pair TRAILS its scores by 3 steps with projection filler
           emitted in between, so the PE queue (strict FIFO) never stalls
           on the exp->mask chain latency (~1.9us).
  normalize: vals_h = acc[0:64] * bcast(1/acc[64])  (PE outer-product bcast,
      reciprocal_approx_fast; deferred into later steps; softmax-drain
      copies run on DVE so ACT carries nothing but the exp stream)
  out_partial = vals^T-chunks @ Wo-rows  -> [1024 q, 1024 D] fp32
All host-staged inputs are laid out so every DMA source row is a multi-KB
contiguous run (1KB packets cap the DMA engines at ~200 GB/s; 8KB packets
saturate ~360 GB/s), and the first-needed tensors issue first across BOTH
HWDGE queues (sync + scalar) so the first kT chain starts ~12us in.
b_v and b_o fold into a host-side constant row (attn rows sum to 1).
"""

import sys
from collections import deque

import numpy as np
import ml_dtypes

if "/opt/trn_rl_repo" not in sys.path:
    sys.path.insert(0, "/opt/trn_rl_repo")

BF = ml_dtypes.bfloat16

B, NKV, NQ, D, H = 4, 2048, 1024, 1024, 16
HD = D // H          # 64
NHL = 8              # heads per core (local)
P = 128
DC = D // P          # 8 contraction chunks over model dim
KC = NKV // P        # 16 key-seq chunks
QT = NQ // 512       # 2 q tiles of 512 for attention
MT = 4               # hd-dim chunks of kT/qT (512/128)

_CACHE = {}


def _build_program():
    import concourse.bass as bass
    import concourse.mybir as mybir
    import concourse.tile as tile
    from concourse import bacc

    f32 = mybir.dt.float32
    bf16 = mybir.dt.bfloat16

    nc = bacc.Bacc(
        "TRN2", target_bir_lowering=False, debug=False, num_devices=8
    )

    # Host staging puts every tensor in the exact SBUF layout so each DMA
    # source ROW is one long contiguous run (-> 8KB packets, ~2x the DMA
    # throughput of the 1KB-segment layouts).
    xT_d = nc.dram_tensor("xT", [4 * P, DC * 512], bf16, kind="ExternalInput").ap()
    yT_d = nc.dram_tensor("yT", [2 * P, DC * 512], bf16, kind="ExternalInput").ap()
    maskT_d = nc.dram_tensor("maskT", [NKV, NQ], bf16, kind="ExternalInput").ap()
    wk_d = nc.dram_tensor("wk", [P, DC * 512], bf16, kind="ExternalInput").ap()
    wv_d = nc.dram_tensor("wv", [P, DC * 512], bf16, kind="ExternalInput").ap()
    wq_d = nc.dram_tensor("wq", [P, DC * 512], bf16, kind="ExternalInput").ap()
    wo_d = nc.dram_tensor("wo", [P, MT * D], bf16, kind="ExternalInput").ap()
    bk_d = nc.dram_tensor("bk", [P, MT], f32, kind="ExternalInput").ap()
    bq_d = nc.dram_tensor("bq", [P, MT], f32, kind="ExternalInput").ap()
    out_d = nc.dram_tensor("out", [NQ, D], bf16, kind="ExternalOutput").ap()

    Exp = mybir.ActivationFunctionType.Exp

    with tile.TileContext(nc) as tc:
        with (
            tc.tile_pool(name="persist", bufs=1) as persist,
            tc.tile_pool(name="work", bufs=3) as work,
            tc.tile_pool(name="empool", bufs=9) as empool,
            tc.tile_pool(name="pmm", bufs=2, space="PSUM") as pmm,
            tc.tile_pool(name="pacc", bufs=2, space="PSUM") as pacc,
            tc.tile_pool(name="psc", bufs=2, space="PSUM") as psc,
        ):
            wk_big = persist.tile([P, DC * 512], bf16, tag="wk", name="wk")
            wv_big = persist.tile([P, DC * 512], bf16, tag="wv", name="wv")
            wq_big = persist.tile([P, DC * 512], bf16, tag="wq", name="wq")
            wo_big = persist.tile([P, MT * D], bf16, tag="wo", name="wo")
            bk_big = persist.tile([P, MT], f32, tag="bk", name="bk")
            bq_big = persist.tile([P, MT], f32, tag="bq", name="bq")
            mT_big = persist.tile([P, KC * NQ], bf16, tag="mT", name="mT")
            xT_big = persist.tile([P, 4 * DC * 512], bf16, tag="xT", name="xT")
            yT_big = persist.tile([P, 2 * DC * 512], bf16, tag="yT", name="yT")

            wk_sb = [wk_big[:, d * 512:(d + 1) * 512] for d in range(DC)]
            wv_sb = [wv_big[:, d * 512:(d + 1) * 512] for d in range(DC)]
            wq_sb = [wq_big[:, d * 512:(d + 1) * 512] for d in range(DC)]
            wo_sb = [wo_big[:, c * D:(c + 1) * D] for c in range(MT)]
            bk_sb = [bk_big[:, m:m + 1] for m in range(MT)]
            bq_sb = [bq_big[:, m:m + 1] for m in range(MT)]
            maskT_sb = [mT_big[:, k * NQ:(k + 1) * NQ] for k in range(KC)]

            def xq(qb, d):   # x^T quarter qb (kseq cols 512qb..), d-chunk
                o = (qb * DC + d) * 512
                return xT_big[:, o:o + 512]

            def yq(nb, d):   # y^T half nb (q cols 512nb..), d-chunk
                o = (nb * DC + d) * 512
                return yT_big[:, o:o + 512]

            # ---- input DMAs: first-needed first, spread across the two
            # HWDGE queues (sync + scalar) so issue costs overlap ----
            def load_xt_q(qb, eng):
                eng.dma_start(
                    xT_big[:, qb * DC * 512:(qb + 1) * DC * 512],
                    xT_d[qb * P:(qb + 1) * P, :],
                )

            def load_yt_h(nb, eng):
                eng.dma_start(
                    yT_big[:, nb * DC * 512:(nb + 1) * DC * 512],
                    yT_d[nb * P:(nb + 1) * P, :],
                )

            def load_mask(k0, k1):
                dst = mT_big.rearrange("p (k c) -> p k c", k=KC)[:, k0:k1, :]
                src = maskT_d.rearrange("(k p) c -> p k c", k=KC, p=P)[
                    :, k0:k1, :]
                nc.sync.dma_start(dst, src)

            # first-needed tensors in halves so the first kT chain's d=0..3
            # matmuls unblock as soon as half the data lands
            HW2 = DC * 512 // 2
            nc.sync.dma_start(xT_big[:, 0:HW2], xT_d[0:P, 0:HW2])
            nc.scalar.dma_start(wk_big[:, 0:HW2], wk_d[:, 0:HW2])
            nc.sync.dma_start(xT_big[:, HW2:2 * HW2], xT_d[0:P, HW2:2 * HW2])
            nc.scalar.dma_start(wk_big[:, HW2:2 * HW2], wk_d[:, HW2:2 * HW2])
            nc.sync.dma_start(wv_big[:, 0:HW2], wv_d[:, 0:HW2])
            nc.sync.dma_start(wv_big[:, HW2:2 * HW2], wv_d[:, HW2:2 * HW2])
            nc.scalar.dma_start(wq_big, wq_d)
            load_yt_h(0, nc.sync)
            nc.scalar.dma_start(bk_big, bk_d)
            nc.scalar.dma_start(bq_big, bq_d)
            load_mask(0, 2)
            nc.scalar.dma_start(
                yT_big[:, DC * 512:2 * DC * 512], yT_d[P:2 * P, :])
            load_xt_q(1, nc.sync)
            load_mask(2, 4)
            load_mask(4, 8)
            load_xt_q(2, nc.sync)
            load_mask(8, 12)
            load_xt_q(3, nc.sync)
            load_mask(12, KC)
            nc.sync.dma_start(wo_big, wo_d)

            ones_sb = persist.tile([1, HD], bf16, tag="ones", name="ones")
            nc.gpsimd.memset(ones_sb, 1.0)

            kT_sb = [
                persist.tile([P, NKV], bf16, tag=f"kT{m}", name=f"kT{m}")
                for m in range(MT)
            ]
            qT_sb = [
                persist.tile([P, NQ], bf16, tag=f"qT{m}", name=f"qT{m}")
                for m in range(MT)
            ]
            v_sb = [
                persist.tile([P, NHL * 65], bf16, tag=f"v{i}", name=f"v{i}")
                for i in range(KC)
            ]
            for i in range(KC):
                nc.gpsimd.memset(
                    v_sb[i].rearrange("p (h c) -> p h c", c=65)[:, :, 64:65], 1.0
                )
            vals_sb = [
                persist.tile([P, NQ], bf16, tag=f"vals{c}", name=f"vals{c}")
                for c in range(MT)
            ]

            # ---- projection op queues (kT/qT chunk m as a list of closures,
            # one matmul each; the chain's last op appends the bias-add) ----
            def proj_ops(m, which, ns=None):
                w_sb, dst, bias, ncols = (
                    (wk_sb, kT_sb, bk_sb, NKV) if which == "k"
                    else (wq_sb, qT_sb, bq_sb, NQ)
                )
                ops = []
                hold = {}
                nlist = list(range(ncols // 512) if ns is None else ns)
                for n in nlist:
                    for d in range(DC):
                        def op(m=m, n=n, d=d, w_sb=w_sb, dst=dst, bias=bias,
                               which=which):
                            if d == 0:
                                hold[n] = pmm.tile(
                                    [P, 512], f32, tag="mm",
                                    name=f"pj{which}{m}_{n}"
                                )
                            nc.tensor.matmul(
                                hold[n],
                                lhsT=w_sb[d][:, m * P:(m + 1) * P],
                                rhs=(xq if which == "k" else yq)(n, d),
                                start=(d == 0),
                                stop=(d == DC - 1),
                            )
                            if d == DC - 1:
                                nc.vector.tensor_scalar_add(
                                    dst[m][:, n * 512:(n + 1) * 512],
                                    hold[n], bias[m]
                                )
                        ops.append(op)
                return ops

            def emit_v(i):
                ps_v = pmm.tile([P, 512], f32, tag="mm", name=f"ps_v{i}")
                for d in range(DC):
                    nc.tensor.matmul(
                        ps_v,
                        lhsT=xq(i // 4, d)[:, (i % 4) * P:(i % 4 + 1) * P],
                        rhs=wv_sb[d],
                        start=(d == 0),
                        stop=(d == DC - 1),
                    )
                v3 = v_sb[i].rearrange("p (h c) -> p h c", c=65)
                nc.vector.tensor_copy(
                    v3[:, :, 0:64], ps_v.rearrange("p (h c) -> p h c", c=64)
                )

            # ---- attention: scores/exp/mask emitted per step; the PV pair
            # trails by 2 steps (filler fills the gap) ----
            pv_backlog = deque()   # entries: (kc, closure)
            norm_pending = deque()

            def make_norm(hp, t, a, h, ut, s_f):
                po = a * HD
                qs = slice(t * 512, (t + 1) * 512)
                # the last block's norms drain in the endgame while both pmm
                # slots hold pre-emitted output-chain heads; their bcast
                # matmuls go to the freed pacc bank instead
                pool, ptag = ((pacc, "acc") if (hp, t) == (3, 1)
                              else (pmm, "mm"))

                def norm_op():
                    r_f = work.tile([1, 512], f32, tag="r", name=f"r{h}_{t}")
                    nc.vector.reciprocal_approx_fast(r_f, s_f)
                    r_b = work.tile([1, 512], bf16, tag="rb", name=f"rb{h}_{t}")
                    nc.vector.tensor_copy(r_b, r_f)
                    bps = pool.tile([HD, 512], f32, tag=ptag,
                                    name=f"bps{h}_{t}")
                    nc.tensor.matmul(
                        bps, lhsT=ones_sb, rhs=r_b, start=True, stop=True
                    )
                    nc.vector.tensor_mul(vals_sb[hp][po:po + HD, qs], ut, bps)
                return norm_op

            def drain_accs(hp, t, accs, h0, h1):
                for a, h in enumerate((h0, h1)):
                    acc = accs[a]
                    # free the PSUM accumulator quickly; all drain copies run
                    # on DVE so the ACT queue carries nothing but exps
                    ut = work.tile(
                        [HD, 512], bf16, tag="ut", name=f"ut{h}_{t}", bufs=4
                    )
                    nc.vector.tensor_copy(ut, acc[0:HD, :])
                    s_f = work.tile(
                        [1, 512], f32, tag="s", name=f"s{h}_{t}", bufs=5
                    )
                    nc.vector.tensor_copy(s_f, acc[64:65, :])
                    norm_pending.append(make_norm(hp, t, a, h, ut, s_f))

            def attn_scores(hp, t, kc, h0, h1, accs_box):
                qs = slice(t * 512, (t + 1) * 512)
                sp2 = psc.tile(
                    [P, 1024], f32, tag="sc", name=f"sp{hp}_{t}_{kc}"
                )
                for a in range(2):
                    po = a * HD
                    nc.tensor.matmul(
                        sp2[:, a * 512:(a + 1) * 512],
                        lhsT=kT_sb[hp][po:po + HD, kc * P:(kc + 1) * P],
                        rhs=qT_sb[hp][po:po + HD, qs],
                        start=True,
                        stop=True,
                    )
                em2 = empool.tile(
                    [P, 1024], bf16, tag="em", name=f"em{hp}_{t}_{kc}"
                )
                nc.scalar.activation(em2, sp2, Exp, scale=0.125)
                mb = (maskT_sb[kc][:, qs]
                      .rearrange("p (o q) -> p o q", o=1)
                      .broadcast_to([P, 2, 512]))
                em3 = em2.rearrange("p (o q) -> p o q", o=2)
                nc.vector.tensor_mul(em3, em3, mb)

                def pv(hp=hp, t=t, kc=kc):
                    if kc == 0:
                        accs_box[:] = [
                            pacc.tile([65, 512], f32, tag="acc",
                                      name=f"acc{h}_{t}")
                            for h in (h0, h1)
                        ]
                    for a, h in enumerate((h0, h1)):
                        nc.tensor.matmul(
                            accs_box[a],
                            lhsT=v_sb[kc][:, h * 65:(h + 1) * 65],
                            rhs=em2[:, a * 512:(a + 1) * 512],
                            start=(kc == 0),
                            stop=(kc == KC - 1),
                        )
                    if kc == KC - 1:
                        drain_accs(hp, t, accs_box, h0, h1)
                pv_backlog.append((kc, pv))

            # ---- deferred projections, drained as per-step filler ----
            pending = {
                1: deque(proj_ops(1, "q", ns=[0])
                         + proj_ops(1, "k", ns=[2, 3])
                         + proj_ops(1, "q", ns=[1])),
                2: deque(proj_ops(2, "k")),
                3: deque(proj_ops(2, "q")),
                4: deque(proj_ops(3, "k", ns=[0, 1, 2])),
                5: deque(proj_ops(3, "q")),
                6: deque(proj_ops(3, "k", ns=[3])),
                7: deque(),
            }

            def pump_pv(depth=2):
                while len(pv_backlog) > depth:
                    pv_backlog.popleft()[1]()

            def wo_units(t2s, pool_pick):
                # output-projection chains; ot tiles are [P, 1024] so the
                # stores are single fat DMAs; PSUM->SBUF copies alternate
                # ACT/DVE; store issues alternate sync/scalar queues
                ops = []
                hold = {}
                for t2 in t2s:
                    for n in range(2):
                        for c in range(MT):
                            def op(t2=t2, n=n, c=c):
                                if c == 0:
                                    pool, tag = pool_pick(t2, n)
                                    hold[(t2, n)] = pool.tile(
                                        [P, 512], f32, tag=tag,
                                        name=f"ps_o{t2}_{n}"
                                    )
                                    if ("ot", t2) not in hold:
                                        hold[("ot", t2)] = work.tile(
                                            [P, 1024], bf16, tag="ot",
                                            name=f"ot{t2}", bufs=3
                                        )
                                        hold[("done", t2)] = 0
                                ps_o = hold[(t2, n)]
                                nc.tensor.matmul(
                                    ps_o,
                                    lhsT=vals_sb[c][:, t2 * P:(t2 + 1) * P],
                                    rhs=wo_sb[c][:, n * 512:(n + 1) * 512],
                                    start=(c == 0),
                                    stop=(c == MT - 1),
                                )
                                if c == MT - 1:
                                    ot = hold[("ot", t2)]
                                    dst = ot[:, n * 512:(n + 1) * 512]
                                    if (t2 + n) % 2 == 0:
                                        nc.scalar.copy(dst, ps_o)
                                    else:
                                        nc.vector.tensor_copy(dst, ps_o)
                                    hold[("done", t2)] += 1
                                    if hold[("done", t2)] == 2:
                                        eng = (nc.sync if t2 % 2 == 0
                                               else nc.scalar)
                                        eng.dma_start(
                                            out_d[t2 * P:(t2 + 1) * P, :], ot
                                        )
                            ops.append(op)
                # group into (t2, n) units of MT c-ops each
                units = []
                for u in range(len(ops) // MT):
                    def unit(u=u):
                        for c in range(MT):
                            ops[u * MT + c]()
                    units.append(unit)
                return units

            # ---- block 0 rides the load window; scores interleave into the
            # projection grind so the exp stream starts early, including 6
            # early (0, t=1) steps whose PVs are stashed until the (0, t=0)
            # accumulators retire (PSUM acc-bank rotation is strictly
            # block-sequential) ----
            wo_first = deque(wo_units(range(0, 4), lambda t2, n: (pmm, "mm")))
            accs00 = []
            for qb in range(4):
                for op in proj_ops(0, "k", ns=[qb]):
                    op()
                for i in range(4 * qb, 4 * qb + 4):
                    emit_v(i)
                    pump_pv(4)
                if qb == 0:
                    for op in proj_ops(0, "q", ns=[0]):
                        op()
                for kc in range(4 * qb, 4 * qb + 2):
                    attn_scores(0, 0, kc, 0, 1, accs00)
                    pump_pv(4)
                if qb < 2:
                    for op in proj_ops(1, "k", ns=[qb]):
                        op()
                for kc in range(4 * qb + 2, 4 * qb + 4):
                    attn_scores(0, 0, kc, 0, 1, accs00)
                    pump_pv(4)
            for op in proj_ops(0, "q", ns=[1]):
                op()
                pump_pv(4)

            # ---- main attention blocks (filler BEFORE scores so chain
            # bias-adds precede the step's mask in the DVE queue) ----
            for hp in range(NHL // 2):
                h0, h1 = 2 * hp, 2 * hp + 1
                for t in (range(QT) if hp > 0 else [1]):
                    blk = 2 * hp + t
                    q = pending[blk]
                    accs = []
                    slots_left = KC
                    for kc in range(KC):
                        n_emit = -(-len(q) // slots_left)  # ceil
                        for _ in range(min(n_emit, len(q))):
                            q.popleft()()
                        slots_left -= 1
                        attn_scores(hp, t, kc, h0, h1, accs)
                        # ready work (mask-complete PVs, vals-ready wo units)
                        # streams ahead of the norm, whose bcast matmul waits
                        # ~1us on the just-emitted DVE reciprocal chain
                        pump_pv(4)
                        if hp == 3 and t == 1 and kc >= 8 and wo_first:
                            wo_first.popleft()()
                        drains = (True if (hp, t) == (3, 1)
                                  else kc % 4 == 2)
                        if drains and (t == 1 or hp == 3) and norm_pending:
                            norm_pending.popleft()()

            while wo_first:
                wo_first.popleft()()
            pump_pv(0)

            # ---- endgame: fused second-half output chains overlap the
            # final norm drains; PE stays dense so HAM stays warm ----
            ps_tail = {}

            def tail_head(t2):
                ps = psc.tile([P, 1024], f32, tag="sc", name=f"ps_o{t2}")
                ot = work.tile([P, 1024], bf16, tag="ot", name=f"ot{t2}",
                               bufs=3)
                ps_tail[t2] = (ps, ot)
                for c in range(3):
                    for n in range(2):
                        nc.tensor.matmul(
                            ps[:, n * 512:(n + 1) * 512],
                            lhsT=vals_sb[c][:, t2 * P:(t2 + 1) * P],
                            rhs=wo_sb[c][:, n * 512:(n + 1) * 512],
                            start=(c == 0), stop=False,
                        )

            def tail_fin(t2):
                ps, ot = ps_tail[t2]
                for n in range(2):
                    nc.tensor.matmul(
                        ps[:, n * 512:(n + 1) * 512],
                        lhsT=vals_sb[3][:, t2 * P:(t2 + 1) * P],
                        rhs=wo_sb[3][:, n * 512:(n + 1) * 512],
                        start=False, stop=True,
                    )
                nc.scalar.copy(ot[:, 0:512], ps[:, 0:512])
                nc.vector.tensor_copy(ot[:, 512:1024], ps[:, 512:1024])
                eng = nc.sync if t2 % 2 == 0 else nc.scalar
                eng.dma_start(out_d[t2 * P:(t2 + 1) * P, :], ot)

            tail_head(4)
            tail_head(5)
            # partial heads for t2=6 on the two pmm slots keep the PE dense
            # through the final norm drain (their c=3 finish comes after)
            ps6 = {}
            for n in range(2):
                ps6[n] = pmm.tile([P, 512], f32, tag="mm", name=f"ps_o6_{n}")
                for c in range(3):
                    nc.tensor.matmul(
                        ps6[n],
                        lhsT=vals_sb[c][:, 6 * P:7 * P],
                        rhs=wo_sb[c][:, n * 512:(n + 1) * 512],
                        start=(c == 0), stop=False,
                    )
            while norm_pending:
                norm_pending.popleft()()
            tail_fin(4)
            tail_fin(5)
            ot6 = work.tile([P, 1024], bf16, tag="ot", name="ot6", bufs=4)
            for n in range(2):
                nc.tensor.matmul(
                    ps6[n],
                    lhsT=vals_sb[3][:, 6 * P:7 * P],
                    rhs=wo_sb[3][:, n * 512:(n + 1) * 512],
                    start=False, stop=True,
                )
                if n == 0:
                    nc.scalar.copy(ot6[:, 0:512], ps6[0])
                else:
                    nc.vector.tensor_copy(ot6[:, 512:1024], ps6[1])
            nc.sync.dma_start(out_d[6 * P:7 * P, :], ot6)
            for u in wo_units(range(7, NQ // P), lambda t2, n: (pmm, "mm")):
                u()

    nc.compile()
    return nc


def _get_program():
    if "nc" not in _CACHE:
        _CACHE["nc"] = _build_program()
    return _CACHE["nc"]


def _per_core_inputs(x, y, mask, W_kv, b_kv, W_q, b_q, W_o):
    """Build the 8 per-core input maps (all staged row-contiguous)."""
    in_maps = []
    mask_f = mask.astype(np.float32)
    for c in range(8):
        b, g = c // 2, c % 2
        gh = np.arange(g * 8, g * 8 + 8)
        k_cols = (gh[:, None] * 2 * HD + np.arange(HD)[None, :]).ravel()
        v_cols = k_cols + HD
        q_cols = slice(g * 512, (g + 1) * 512)
        xb = np.ascontiguousarray(x[b].T).astype(BF)       # [D, NKV]
        yb = np.ascontiguousarray(y[b].T).astype(BF)       # [D, NQ]
        # quarter-major: row (qb*128+p), col (d*512+c) = xb[d*128+p, qb*512+c]
        xq = (xb.reshape(DC, P, 4, 512).transpose(2, 1, 0, 3)
              .reshape(4 * P, DC * 512))
        yh = (yb.reshape(DC, P, 2, 512).transpose(2, 1, 0, 3)
              .reshape(2 * P, DC * 512))

        def wstage(w):  # [1024, 512] -> [128, 8*512] row-contiguous
            return np.ascontiguousarray(
                w.reshape(DC, P, 512).transpose(1, 0, 2).reshape(P, DC * 512)
            ).astype(BF)

        wo_st = np.ascontiguousarray(
            W_o[q_cols, :].reshape(MT, P, D).transpose(1, 0, 2)
            .reshape(P, MT * D)
        ).astype(BF)
        in_maps.append({
            "xT": np.ascontiguousarray(xq),
            "yT": np.ascontiguousarray(yh),
            "maskT": np.ascontiguousarray(mask_f[b].T).astype(BF),
            "wk": wstage(W_kv[:, k_cols]),
            "wv": wstage(W_kv[:, v_cols]),
            "wq": wstage(W_q[:, q_cols]),
            "wo": wo_st,
            "bk": np.ascontiguousarray(
                b_kv[k_cols].astype(np.float32).reshape(MT, P).T),
            "bq": np.ascontiguousarray(
                b_q[np.arange(g * 512, (g + 1) * 512)]
                .astype(np.float32).reshape(MT, P).T),
        })
    return in_maps


def kernel(x, y, mask, W_kv, b_kv, W_q, b_q, W_o, b_o):
    from concourse import bass_utils

    x = np.asarray(x, np.float32)
    y = np.asarray(y, np.float32)
    mask = np.asarray(mask)
    W_kv = np.asarray(W_kv, np.float32)
    b_kv = np.asarray(b_kv, np.float32)
    W_q = np.asarray(W_q, np.float32)
    b_q = np.asarray(b_q, np.float32)
    W_o = np.asarray(W_o, np.float32)
    b_o = np.asarray(b_o, np.float32)

    nc = _get_program()
    in_maps = _per_core_inputs(x, y, mask, W_kv, b_kv, W_q, b_q, W_o)
    res = bass_utils.run_bass_kernel_spmd(nc, in_maps, core_ids=list(range(8)))

    # b_v folds into a constant row: attn rows sum to 1, so each head adds
    # b_v_h @ W_o_h to every output row; b_o adds on top.
    v_cols_all = (np.arange(H)[:, None] * 2 * HD + HD
                  + np.arange(HD)[None, :]).ravel()
    const_row = b_kv[v_cols_all].astype(np.float32) @ W_o + b_o

    out = np.empty((B, NQ, D), np.float32)
    for b in range(B):
        out[b] = (res.results[2 * b]["out"].astype(np.float32)
                  + res.results[2 * b + 1]["out"].astype(np.float32)
                  + const_row)
    return out


if __name__ == "__main__":
    import reference

    inputs = {k: np.asarray(v) for k, v in reference.setup_inputs().items()}
    got = kernel(**inputs)
    exp = np.asarray(reference.reference(**inputs))
    err = np.abs(got - exp)
    print("absmax rel err:", err.max() / np.abs(exp).max())
